# revision 52
# baseline (speedup 1.0000x reference)
"""Tensor-parallel (over GQA head groups) multi-head attention for 8 trn2 cores.

Each core owns 4 query heads + their shared kv head (one GQA group), the
matching 384 rows of wqkv and 256 columns of wo.  Every core computes a full
[S, D] partial of the output projection; the host sums the 8 partials.

v2: fp16 data end-to-end (host converts; PSUM stays fp32) and a software-
pipelined schedule that keeps the PE busy continuously:
  - projection runs in four 512-column quarters, each as three m-serial
    passes (q01/q23/kv) over 16 resident hT chunks -> only 2 PSUM banks,
    so projection overlaps attention in PSUM;
  - attention chunk c is emitted between projection quarters c+1/c+2;
    out-projection chunks are emitted after the projection pools close;
  - scores on diagonal blocks restrict the moving operand to the live
    triangle columns (128-granular staircase);
  - the softmax denominator broadcast reuses the evacuated po psum bank
    (ones-row matmul per bank, then a lane-aligned reciprocal);
  - exp is computed as exp(s - 4) so the unnormalized f16 sums stay in
    range (softmax is shift-invariant);
  - v is kept in f32 (second half of the kv evacuation) and transposed
    f32r through the projection psum ring.
Dataflow inside one core otherwise as v1 (scores transposed [ks, qs], ones
column in V for the denominator).
"""

import sys

if "/opt/trn_rl_repo" not in sys.path:
    sys.path.insert(0, "/opt/trn_rl_repo")

import numpy as np

S = 2048
D = 2048
HD = 64
N_HEAD = 32
N_KV = 8
NCORES = 8
QH_PER_CORE = N_HEAD // NCORES  # 4
KV_SIZE = N_KV * HD  # 512

_CACHE = {}


def _build_module(debug=False):
    from contextlib import ExitStack

    import concourse.mybir as mybir
    import concourse.tile as tile
    from concourse import bacc
    from concourse.bass import ds

    FP = mybir.dt.float32
    F16 = mybir.dt.float16
    EXP = mybir.ActivationFunctionType.Exp

    nc = bacc.Bacc(
        "TRN2",
        target_bir_lowering=False,
        debug=False,
        enable_asserts=False,
        num_devices=NCORES,
    )
    # register the exp bias constant (softmax shift, see emit_expav)
    _c = nc.alloc_sbuf_tensor("const-float32-neg4", [128, 1], FP)
    nc.gpsimd.memset(_c.ap(), -4.0)
    nc.const_aps.aps[(FP, -4.0)] = _c.ap()
    nc.all_engine_barrier()

    # [p, 2048*g + s] = hidden[s, 128*g + p]
    hT = nc.dram_tensor("hT", [128, 16 * S], F16, kind="ExternalInput").ap()
    # [p, 384*g + r] = wqkv_local[r, 128*g + p]; r: 0-255 q, 256-319 k, 320-383 v
    wq = nc.dram_tensor("wq", [128, 16 * 384], F16, kind="ExternalInput").ap()
    # [p, 2048*u + e] = wo[e, 256*core + 128*u + p]
    wo = nc.dram_tensor("wo", [128, 2 * 2048], F16, kind="ExternalInput").ap()
    # rope tables, full scale (the q weights carry 1/sqrt(hd)):
    # cols 0:2048 cos, 2048:4096 sin'
    rqq = nc.dram_tensor("rqq", [128, 2 * S], F16, kind="ExternalInput").ap()
    # cols 0:128 tri[p, f] = (p <= f); cols 128:256 identity[p, f] = (p == f)
    tri = nc.dram_tensor("tri", [128, 256], F16, kind="ExternalInput").ap()
    # f32 identity in rows 64-127 (rhs of the f32r v transposes)
    idf = nc.dram_tensor("idf", [128, 64], mybir.dt.float32r, kind="ExternalInput").ap()
    out = nc.dram_tensor("out", [S, D], F16, kind="ExternalOutput").ap()
    if debug:
        dbg = {
            name: nc.dram_tensor(f"dbg_{name}", shape, F16, kind="ExternalOutput").ap()
            for name, shape in (
                ("q01s", [128, S]),
                ("q23s", [128, S]),
                ("ks", [128, S]),
                ("v65", [128, 16 * 65]),
                ("o2a", [128, S]),
                ("o2b", [128, S]),
                ("ex0", [128, 8 * 1024]),
                ("sc0", [128, 8 * 1024]),
                ("po0", [65, 1024]),
                ("poc0", [65, 1024]),
                ("rbr0", [64, 1024]),
            )
        }

    with tile.TileContext(nc) as tc, ExitStack() as ctx:
        const = ctx.enter_context(tc.tile_pool(name="const", bufs=1))
        wqt = [
            const.tile([128, 4 * 384], F16, tag=f"wq{k}", name=f"wq_sb{k}")
            for k in range(4)
        ]

        def wq_ap(g, m):
            return wqt[g // 4][:, ds(384 * (g % 4) + 128 * m, 128)]

        F32R = mybir.dt.float32r
        wo_sb = const.tile([128, 4096], F16, tag="wo")
        rqq_sb = const.tile([128, 4096], F16, tag="rqq")
        tri_sb = const.tile([128, 256], F16, tag="tri")
        idf_sb = const.tile([128, 64], FP, tag="idf")
        ident64 = idf_sb[64:128, :].bitcast(F32R)

        qsw = ctx.enter_context(tc.tile_pool(name="qsw", bufs=1))
        q01s = qsw.tile([128, S], F16, tag="q01s")
        q23s = qsw.tile([128, S], F16, tag="q23s")
        ks = qsw.tile([128, S], F16, tag="ks")
        v_sb = qsw.tile([128, 16 * 65], F16, tag="v")
        o2a = qsw.tile([128, S], F16, tag="o2a")
        o2b = qsw.tile([128, S], F16, tag="o2b")
        v65 = v_sb.rearrange("p (j c) -> p j c", c=65)

        # attention pools (live through the whole kernel)
        expool = ctx.enter_context(tc.tile_pool(name="ex", bufs=4))
        rspool = ctx.enter_context(tc.tile_pool(name="rs", bufs=2))
        rcpool = ctx.enter_context(tc.tile_pool(name="rc", bufs=2))
        rbpool = ctx.enter_context(tc.tile_pool(name="rb", bufs=2))
        nmpool = ctx.enter_context(tc.tile_pool(name="nm", bufs=2))
        # attention psum pools are created after quarter 0 releases its
        # 3-bank ring (right side of the arena; release is LIFO per side)
        phaseA = ExitStack()
        psS = psO = None

        # projection-phase pools (close before the out-projection opens)
        phaseP = ExitStack()
        hpool = phaseP.enter_context(tc.tile_pool(name="hp", bufs=24))
        qraw = phaseP.enter_context(tc.tile_pool(name="qraw", bufs=1))
        q01 = qraw.tile([128, S], F16, tag="q01")
        q23 = qraw.tile([128, S], F16, tag="q23")
        kv = qraw.tile([128, S], F16, tag="kv")  # rows 0:64 = k (rope input)
        vraw = qraw.tile([128, S], FP, tag="vraw")  # rows 64:128 = v, f32
        scpool = phaseP.enter_context(tc.tile_pool(name="sc", bufs=4))
        phase0 = ExitStack()
        psA0 = phase0.enter_context(tc.tile_pool(name="psA0", bufs=1, space="PSUM"))
        psA = None

        SWAP_MASK = [i ^ 1 for i in range(32)]

        def rope_quarter(dst, raw, p, costab, sintab, q, nm):
            # dst = raw * cos + pairswap(raw) * sin' on [0:p, 512q:512q+512]
            cs = ds(512 * q, 512)
            sw = scpool.tile([128, 512], F16, tag="sc", name=f"sw_{nm}{q}")
            nc.vector.stream_shuffle(sw[0:p, :], raw[0:p, cs], SWAP_MASK)
            t0 = scpool.tile([128, 512], F16, tag="sc", name=f"t0_{nm}{q}")
            nc.vector.tensor_mul(t0[0:p, :], raw[0:p, cs], costab[0:p, cs])
            nc.vector.tensor_mul(sw[0:p, :], sw[0:p, :], sintab[0:p, cs])
            nc.vector.tensor_add(dst[0:p, cs], t0[0:p, :], sw[0:p, :])

        # global DMA plan: one ordered stream of large transfers.  hT comes as
        # 2048-column quads ([128, 4, 512] strided, >=1KB contiguous runs) so
        # each DMA is bus-bound, not HWDGE-dispatch-bound (625ns fixed cost).
        hT_r = hT.rearrange("p (g s) -> p g s", s=2048)
        rq_r = rqq.rearrange("p (h s) -> p h s", s=2048)
        hq_tiles = {}

        def dma_hq(q, t, half=None):
            # quad t of quarter q: g-chunks 4t..4t+3, columns 512q..512q+512
            if half is None:
                gs, n = 4 * t, 4
            else:
                gs, n = 4 * t + 2 * half, 2
            tile = hq_tiles.get((q, t))
            if tile is None:
                tile = hpool.tile([128, 4, 512], F16, tag="hc", name=f"hq_{q}_{t}")
                hq_tiles[(q, t)] = tile
            nc.sync.dma_start(
                tile[:, gs - 4 * t : gs - 4 * t + n, :],
                hT_r[:, gs : gs + n, ds(512 * q, 512)],
            )

        def hc_ap(q, g):
            return hq_tiles[(q, g // 4)][:, g % 4, :]

        rqsb_r = rqq_sb.rearrange("p (h s) -> p h s", s=2048)

        def dma_rq(q):
            nc.sync.dma_start(
                rqsb_r[:, :, ds(512 * q, 512)], rq_r[:, :, ds(512 * q, 512)]
            )

        dma_plan = [
            ("wq0a", lambda: nc.sync.dma_start(wqt[0][:, 0:768], wq[:, 0:768])),
            ("hq00a", lambda: dma_hq(0, 0, 0)),
            ("wq0b", lambda: nc.sync.dma_start(wqt[0][:, 768:1536], wq[:, 768:1536])),
            ("hq00b", lambda: dma_hq(0, 0, 1)),
            ("wq1", lambda: nc.sync.dma_start(wqt[1][:], wq[:, ds(1536, 1536)])),
            ("hq01", lambda: dma_hq(0, 1)),
            ("wq2", lambda: nc.sync.dma_start(wqt[2][:], wq[:, ds(3072, 1536)])),
            ("hq02", lambda: dma_hq(0, 2)),
            ("wq3", lambda: nc.sync.dma_start(wqt[3][:], wq[:, ds(4608, 1536)])),
            ("rq0", lambda: dma_rq(0)),
            ("tri", lambda: (nc.sync.dma_start(tri_sb[:], tri),
                             nc.sync.dma_start(idf_sb[:].bitcast(F32R), idf))),
            ("hq03", lambda: dma_hq(0, 3)),
            ("hq10", lambda: dma_hq(1, 0)),
            ("hq11", lambda: dma_hq(1, 1)),
            ("hq12", lambda: dma_hq(1, 2)),
            ("rq1", lambda: dma_rq(1)),
            ("hq13", lambda: dma_hq(1, 3)),
            ("hq20", lambda: dma_hq(2, 0)),
            ("wo0", lambda: nc.sync.dma_start(wo_sb[:, 0:2048], wo[:, 0:2048])),
            ("hq21", lambda: dma_hq(2, 1)),
            ("hq22", lambda: dma_hq(2, 2)),
            ("wo1", lambda: nc.sync.dma_start(wo_sb[:, 2048:4096], wo[:, 2048:4096])),
            ("rq2", lambda: dma_rq(2)),
            ("hq23", lambda: dma_hq(2, 3)),
            ("hq30", lambda: dma_hq(3, 0)),
            ("hq31", lambda: dma_hq(3, 1)),
            ("hq32", lambda: dma_hq(3, 2)),
            ("rq3", lambda: dma_rq(3)),
            ("hq33", lambda: dma_hq(3, 3)),
        ]
        plan_pos = {key: idx for idx, (key, _) in enumerate(dma_plan)}
        feed_state = {"next": 0}

        def feed_until(key):
            stop = plan_pos[key] + 1
            while feed_state["next"] < stop:
                dma_plan[feed_state["next"]][1]()
                feed_state["next"] += 1

        def emit_vtrans(q, vtp):
            # v transposes for quarter q: f32r through the projection psum
            # ring slot, then one f16 evacuation into v65
            vt16 = None
            vt = vtp.tile([128, 512], FP, tag=vtp_tag(vtp), name=f"vt_{q}")
            for jj in range(4):
                j = 4 * q + jj
                nc.tensor.transpose(
                    vt[:, ds(64 * jj, 64)].bitcast(F32R),
                    vraw[64:128, ds(128 * j, 128)].bitcast(F32R),
                    ident64,
                )
            nc.vector.tensor_copy(
                v65[:, 4 * q : 4 * q + 4, 0:64],
                vt[:, 0:256].rearrange("p (jj c) -> p jj c", c=64),
            )
            nc.vector.tensor_copy(
                v65[:, 4 * q : 4 * q + 4, 64:65],
                tri_sb[:, 127:128][:, None, :].to_broadcast([128, 4, 1]),
            )

        def vtp_tag(pool):
            return "p0" if pool is psA0 else "pj"

        def gen_proj_quarter0():
            # quarter 0 runs all three m-passes interleaved at the pace the
            # DMA stream can sustain (the front is inherently supply-bound)
            pts = [
                psA0.tile([128, 512], FP, tag=f"p{m}", name=f"pj0_{m}")
                for m in range(3)
            ]
            feed_until("hq00b")
            for g in range(12):
                feed_until(dma_plan[min(g + 4, 13)][0])
                for m in range(3):
                    nc.tensor.matmul(
                        pts[m][:], wq_ap(g, m), hc_ap(0, g),
                        start=(g == 0), stop=False,
                    )
                yield 639
            # finish the three passes m-serially, kv first: rope-k (which
            # gates the first attention scores) starts ~3us earlier
            cos_t, sin_t = rqq_sb[:, 0:2048], rqq_sb[:, 2048:4096]
            for m, dst, dsts, p, nm in (
                (2, kv, ks, 64, "k"),
                (0, q01, q01s, 128, "q01"),
                (1, q23, q23s, 128, "q23"),
            ):
                feed_until("hq11")
                for g in range(12, 16):
                    nc.tensor.matmul(
                        pts[m][:], wq_ap(g, m), hc_ap(0, g),
                        start=False, stop=(g == 15),
                    )
                if m == 2:
                    nc.scalar.copy(kv[0:64, 0:512], pts[m][0:64, :])
                    nc.scalar.copy(vraw[64:128, 0:512].bitcast(F32R), pts[m][64:128, :])
                else:
                    nc.scalar.copy(dst[:, 0:512], pts[m][:])
                rope_quarter(dsts, dst, p, cos_t, sin_t, 0, nm)
                if m == 2:
                    nc.sync.dma_start(ks[64:128, 0:512], ks[0:64, 0:512])
                yield 852
            emit_vtrans(0, psA0)
            yield 80
            feed_until("hq11")
            yield 120

        def gen_proj_quarter(q):
            feed_until(f"hq{q}1")
            pts = {}
            for m in (0, 1):
                pts[m] = psA.tile([128, 512], FP, tag="pj", name=f"pj_{q}_{m}")
            for g in range(16):
                if g % 4 == 2 and g < 12:
                    feed_until(f"hq{q}{g // 4 + 2}" if g // 4 + 2 <= 3 else f"hq{q}3")
                for m in (0, 1):
                    nc.tensor.matmul(
                        pts[m][:], wq_ap(g, m), hc_ap(q, g),
                        start=(g == 0), stop=(g == 15),
                    )
                yield 426
            cs = ds(512 * q, 512)
            nc.scalar.copy(q01[:, cs], pts[0][:])
            rope_quarter(q01s, q01, 128, rqq_sb[:, 0:2048], rqq_sb[:, 2048:4096], q, "q01")
            yield 300
            nc.scalar.copy(q23[:, cs], pts[1][:])
            rope_quarter(q23s, q23, 128, rqq_sb[:, 0:2048], rqq_sb[:, 2048:4096], q, "q23")
            yield 300
            pt2 = psA.tile([128, 512], FP, tag="pj", name=f"pj_{q}_2")
            for g in range(16):
                nc.tensor.matmul(
                    pt2[:], wq_ap(g, 2), hc_ap(q, g),
                    start=(g == 0), stop=(g == 15),
                )
                if g % 4 == 3:
                    # deep-prefetch the next quarter while the psum ring is
                    # the only DMA consumer
                    if q < 3:
                        feed_until(f"hq{q + 1}{min(g // 4, 3)}")
                    yield 852
            nc.scalar.copy(kv[0:64, cs], pt2[0:64, :])
            nc.scalar.copy(vraw[64:128, cs].bitcast(F32R), pt2[64:128, :])
            rope_quarter(ks, kv, 64, rqq_sb[:, 0:2048], rqq_sb[:, 2048:4096], q, "k")
            yield 300
            # duplicate rotated k at partitions 64-127 (odd heads' score
            # matmuls read lhsT/rhs both at base 64)
            nc.sync.dma_start(ks[64:128, cs], ks[0:64, cs])
            emit_vtrans(q, psA)
            if q < 3:
                feed_until(f"hq{q + 1}1")
            yield 120

        def gen_attention_chunk(c):
            nj = 4 * c + 4
            for hp in range(2):
                po = psO.tile([65, 1024], FP, tag="po", name=f"po_{c}_{hp}")

                def emit_scores(j):
                    r = j - 4 * c  # >= 0 on diagonal blocks
                    off = 128 * r if r >= 0 else 0
                    ps = psS.tile([128, 1024], FP, tag="ps", name=f"ps_{c}_{hp}_{j}")
                    for hh in range(2):
                        h = 2 * hp + hh
                        qt = q01s if h < 2 else q23s
                        base = 64 * (h % 2)
                        nc.tensor.matmul(
                            ps[:, ds(512 * hh + off, 512 - off)],
                            ks[base : base + 64, ds(128 * j, 128)],
                            qt[base : base + 64, ds(512 * c + off, 512 - off)],
                        )
                    return ps, off, r >= 0

                def emit_expav(j, ps, off, diag):
                    # exp(s - 4): softmax is shift-invariant and the bias
                    # keeps the unnormalized f16 sums (up to ~exp(9) * |v|)
                    # well inside f16 range
                    ex = expool.tile([128, 1024], F16, tag="ex", name=f"ex_{c}_{hp}_{j}")
                    if not diag:
                        nc.scalar.activation(ex[:], ps[:], EXP, bias=-4.0)
                    else:
                        w = 512 - off
                        psv = ps.rearrange("p (h w) -> p h w", w=512)[:, :, ds(off, w)]
                        exv = ex.rearrange("p (h w) -> p h w", w=512)[:, :, ds(off, w)]
                        nc.scalar.activation(exv, psv, EXP, bias=-4.0)
                        exd = ex.rearrange("p (h w) -> p h w", w=512)[:, :, ds(off, 128)]
                        nc.vector.tensor_mul(
                            exd,
                            exd,
                            tri_sb[:, 0:128][:, None, :].to_broadcast([128, 2, 128]),
                        )
                    if debug and c == 0:
                        sl = ds(1024 * (4 * hp + j), 1024)
                        nc.sync.dma_start(dbg["ex0"][:, sl], ex[:])
                        sc16 = expool.tile(
                            [128, 1024], F16, tag="ex", name=f"scd_{hp}_{j}"
                        )
                        nc.vector.tensor_copy(sc16[:], ps[:])
                        nc.sync.dma_start(dbg["sc0"][:, sl], sc16[:])
                    for hh in range(2):
                        nc.tensor.matmul(
                            po[0:65, ds(512 * hh + off, 512 - off)],
                            v_sb[:, ds(65 * j, 65)],
                            ex[:, ds(512 * hh + off, 512 - off)],
                            start=(j == 0),
                            stop=(j == nj - 1),
                            skip_group_check=True,
                        )

                # one-j lookahead: scores(j+1) land on the PE between
                # scores(j) and av(j) so the exp never stalls the PE
                def jcost(j):
                    # PE ns of one scores OR av pair at this block's trim
                    r = j - 4 * c
                    off = 128 * r if r >= 0 else 0
                    return int((512 - off) * 0.833)

                prev = emit_scores(0)
                for j in range(1, nj):
                    cur = emit_scores(j)
                    emit_expav(j - 1, *prev)
                    prev = cur
                    yield jcost(j) + jcost(j - 1)
                emit_expav(nj - 1, *prev)
                yield jcost(nj - 1)
                # evacuate the accumulator so the bank frees for the next
                # head pair, then normalize: reciprocal of the sums row,
                # partition_broadcast on the (idle) Pool engine, two muls
                poc = rspool.tile([65, 1024], F16, tag="rs", name=f"poc_{c}_{hp}")
                if debug and c == 0 and hp == 0:
                    pod = rspool.tile([65, 1024], F16, tag="pod", name="pod")
                    nc.vector.tensor_copy(pod[:], po[:])
                    nc.sync.dma_start(dbg["po0"], pod[:])
                nc.vector.tensor_copy(poc[:, 0:512], po[:, 0:512])
                nc.scalar.copy(poc[:, 512:1024], po[:, 512:1024])
                rbr = rbpool.tile([64, 1024], F16, tag="rbr", name=f"rbr_{c}_{hp}")
                dsttile = o2a if hp == 0 else o2b
                nm = nmpool.tile([64, 512], F16, tag="nm", name=f"nm_{c}_{hp}")
                # broadcast the sums row back into the (already-evacuated)
                # po bank with a ones-row matmul -- ones at base 64 to match
                # poc's denominator row -- then a lane-aligned reciprocal
                for half in range(2):
                    hs = ds(512 * half, 512)
                    nc.tensor.matmul(
                        po[0:64, hs], tri_sb[64:65, 64:128], poc[64:65, hs],
                        start=True, stop=True,
                    )
                    with nc.allow_low_precision(reason="softmax denom recip f16"):
                        nc.vector.reciprocal(rbr[0:64, hs], po[0:64, hs])
                    if half == 0:
                        nc.vector.tensor_mul(
                            dsttile[0:64, ds(512 * c, 512)],
                            poc[0:64, hs],
                            rbr[0:64, hs],
                        )
                    else:
                        nc.vector.tensor_mul(nm[0:64, :], poc[0:64, hs], rbr[0:64, hs])
                nc.sync.dma_start(dsttile[64:128, ds(512 * c, 512)], nm[0:64, :])
                if debug and c == 0 and hp == 0:
                    nc.sync.dma_start(dbg["poc0"], poc[:])
                    nc.sync.dma_start(dbg["rbr0"], rbr[0:64, :])
                yield 60

        post = {}

        def open_post_pools():
            post["ost"] = ctx.enter_context(tc.tile_pool(name="ost", bufs=6))
            post["psP"] = ctx.enter_context(tc.tile_pool(name="psP", bufs=2, space="PSUM"))

        def gen_outproj_chunk(c, tail=False, pskey="psP", bs=range(4)):
            for b in bs:
                for n2 in range(2):  # pairs of 512-wide e-slices -> one DMA
                    st = post["ost"].tile(
                        [128, 1024], F16, tag="st", name=f"st_{c}_{b}_{n2}"
                    )
                    for nn in range(2):
                        n = 2 * n2 + nn
                        pp = post[pskey].tile(
                            [128, 512], FP, tag="pp", name=f"pp_{c}_{b}_{n}"
                        )
                        nc.tensor.matmul(
                            pp[:],
                            o2a[:, ds(512 * c + 128 * b, 128)],
                            wo_sb[:, ds(512 * n, 512)],
                            start=True,
                            stop=False,
                        )
                        nc.tensor.matmul(
                            pp[:],
                            o2b[:, ds(512 * c + 128 * b, 128)],
                            wo_sb[:, ds(2048 + 512 * n, 512)],
                            start=False,
                            stop=True,
                        )
                        # in the pure-PE tail alternate evacuation engines so
                        # the psum ring keeps pace with the matmuls
                        if tail and nn == 1:
                            nc.scalar.copy(st[:, ds(512, 512)], pp[:])
                        else:
                            nc.vector.tensor_copy(st[:, ds(512 * nn, 512)], pp[:])
                        yield 426
                    nc.sync.dma_start(
                        out[ds(128 * (4 * c + b), 128), ds(1024 * n2, 1024)], st[:]
                    )

        def chain(*gens):
            for g in gens:
                yield from g

        def closer():
            phaseP.close()
            open_post_pools()
            return
            yield  # pragma: no cover

        def weave(ga, gb, wa=1.0, wb=1.0):
            # proportional-progress interleave of two emission streams:
            # step the stream with the smaller fraction-complete so a short
            # filler spreads across the whole window instead of front-loading
            ta = tb = 0.0
            da = db = False
            while not (da and db):
                if db or (not da and ta / wa <= tb / wb):
                    try:
                        ta += next(ga)
                    except StopIteration:
                        da = True
                else:
                    try:
                        tb += next(gb)
                    except StopIteration:
                        db = True

        def run(g):
            for _ in g:
                pass

        # ---- pipeline: P0 [P1|A0] [P2|A1] [P3,close,O0|A2] [O1,O2|A3] O3 --
        run(gen_proj_quarter0())
        phase0.close()
        psA = phaseP.enter_context(tc.tile_pool(name="psA", bufs=2, space="PSUM"))
        psS = phaseA.enter_context(
            tc.tile_pool(name="psS", bufs=2, space="PSUM", side="right")
        )
        psO = phaseA.enter_context(
            tc.tile_pool(name="psO", bufs=1, space="PSUM", side="right")
        )
        weave(gen_proj_quarter(1), gen_attention_chunk(0))
        weave(gen_proj_quarter(2), gen_attention_chunk(1))
        weave(
            chain(gen_proj_quarter(3), closer(), gen_outproj_chunk(0)),
            gen_attention_chunk(2),
            wa=17.5,
            wb=16.6,
        )
        weave(
            chain(gen_outproj_chunk(1), gen_outproj_chunk(2)),
            gen_attention_chunk(3),
            wa=13.6,
            wb=22.1,
        )
        # attention psum freed -> deep out-proj ring for the pure-PE tail
        phaseA.close()
        post["psP2"] = ctx.enter_context(
            tc.tile_pool(name="psP2", bufs=4, space="PSUM", side="right")
        )
        run(gen_outproj_chunk(3, tail=True, pskey="psP2"))
        if debug:
            for name, tile in (
                ("q01s", q01s), ("q23s", q23s), ("ks", ks),
                ("v65", v_sb), ("o2a", o2a), ("o2b", o2b),
            ):
                nc.sync.dma_start(dbg[name], tile[:])

    nc.compile()
    return nc


def get_module(debug=False):
    key = ("nc", debug)
    if key not in _CACHE:
        _CACHE[key] = _build_module(debug=debug)
    return _CACHE[key]


def _pack16(x):
    # [16*128, N] -> [128, 16*N] with [p, N*g + n] = x[128*g + p, n]
    n = x.shape[1]
    return (
        np.ascontiguousarray(x.reshape(16, 128, n).transpose(1, 0, 2)).reshape(128, 16 * n)
    )


def prep_inputs(hidden_states, freqs_cis, wqkv, wo):
    h = np.asarray(hidden_states, dtype=np.float32)[0]  # [S, D]
    fc = np.asarray(freqs_cis, dtype=np.float32)  # [S, 32, 2]
    wqkv = np.asarray(wqkv, dtype=np.float32)  # [3072, D]
    wo = np.asarray(wo, dtype=np.float32)  # [D, D]

    hT_sb = _pack16(np.ascontiguousarray(h.T)).astype(np.float16)

    cos = fc[:, :, 0]  # [S, 32]
    sin = fc[:, :, 1]
    cos_ext = np.repeat(cos, 2, axis=1).T  # [64, S]
    sgn = np.where(np.arange(HD) % 2 == 0, -1.0, 1.0).astype(np.float32)[:, None]
    sin_ext = np.repeat(sin, 2, axis=1).T * sgn  # sin'[d, s]
    rqq_np = np.concatenate(
        [np.tile(cos_ext, (2, 1)), np.tile(sin_ext, (2, 1))], axis=1
    ).astype(np.float16)  # [128, 4096] full scale
    idf_np = np.zeros((128, 64), dtype=np.float32)
    idf_np[64:128] = np.eye(64, dtype=np.float32)
    tri_np = np.concatenate(
        [
            (np.arange(128)[:, None] <= np.arange(128)[None, :]).astype(np.float16),
            np.eye(128, dtype=np.float16),
        ],
        axis=1,
    )  # [128, 256]: triangle | identity

    in_maps = []
    for i in range(NCORES):
        scale = 1.0 / np.sqrt(np.float32(HD))
        wl = np.concatenate(
            [
                wqkv[256 * i : 256 * i + 256] * scale,
                wqkv[D + 64 * i : D + 64 * i + 64],
                wqkv[D + KV_SIZE + 64 * i : D + KV_SIZE + 64 * i + 64],
            ],
            axis=0,
        )  # [384, D]
        wq_sb = _pack16(np.ascontiguousarray(wl.T)).astype(np.float16)
        woT = np.ascontiguousarray(wo[:, 256 * i : 256 * i + 256].T)  # [256, D]
        wo_sb = (
            np.ascontiguousarray(woT.reshape(2, 128, D).transpose(1, 0, 2))
            .reshape(128, 2 * D)
            .astype(np.float16)
        )
        in_maps.append(
            {
                "hT": hT_sb,
                "wq": wq_sb,
                "wo": wo_sb,
                "rqq": rqq_np,
                "tri": tri_np,
                "idf": idf_np,
            }
        )
    return in_maps


def run_on_hw(in_maps, trace=False, **kw):
    from concourse.bass_utils import run_bass_kernel_spmd

    nc = get_module()
    return run_bass_kernel_spmd(nc, in_maps, list(range(NCORES)), trace=trace, **kw)


def kernel(hidden_states, freqs_cis, wqkv, wo):
    in_maps = prep_inputs(hidden_states, freqs_cis, wqkv, wo)
    res = run_on_hw(in_maps)
    acc = np.zeros((S, D), dtype=np.float64)
    for r in res.results:
        acc += np.asarray(r["out"], dtype=np.float64)
    return acc.astype(np.float32).reshape(1, S, D)


# revision 53
# speedup vs baseline: 1.0077x; 1.0077x over previous
"""Tensor-parallel (over GQA head groups) multi-head attention for 8 trn2 cores.

Each core owns 4 query heads + their shared kv head (one GQA group), the
matching 384 rows of wqkv and 256 columns of wo.  Every core computes a full
[S, D] partial of the output projection; the host sums the 8 partials.

v2: fp16 data end-to-end (host converts; PSUM stays fp32) and a software-
pipelined schedule that keeps the PE busy continuously:
  - projection runs in four 512-column quarters, each as three m-serial
    passes (q01/q23/kv) over 16 resident hT chunks -> only 2 PSUM banks,
    so projection overlaps attention in PSUM;
  - attention chunk c is emitted between projection quarters c+1/c+2;
    out-projection chunks are emitted after the projection pools close;
  - scores on diagonal blocks restrict the moving operand to the live
    triangle columns (128-granular staircase);
  - the softmax denominator broadcast reuses the evacuated po psum bank
    (ones-row matmul per bank, then a lane-aligned reciprocal);
  - exp is computed as exp(s - 4) so the unnormalized f16 sums stay in
    range (softmax is shift-invariant);
  - v is kept in f32 (second half of the kv evacuation) and transposed
    f32r through the projection psum ring.
Dataflow inside one core otherwise as v1 (scores transposed [ks, qs], ones
column in V for the denominator).
"""

import sys

if "/opt/trn_rl_repo" not in sys.path:
    sys.path.insert(0, "/opt/trn_rl_repo")

import numpy as np

S = 2048
D = 2048
HD = 64
N_HEAD = 32
N_KV = 8
NCORES = 8
QH_PER_CORE = N_HEAD // NCORES  # 4
KV_SIZE = N_KV * HD  # 512

_CACHE = {}


def _build_module(debug=False):
    from contextlib import ExitStack

    import concourse.mybir as mybir
    import concourse.tile as tile
    from concourse import bacc
    from concourse.bass import ds

    FP = mybir.dt.float32
    F16 = mybir.dt.float16
    EXP = mybir.ActivationFunctionType.Exp

    nc = bacc.Bacc(
        "TRN2",
        target_bir_lowering=False,
        debug=False,
        enable_asserts=False,
        num_devices=NCORES,
    )
    # register the exp bias constant (softmax shift, see emit_expav)
    _c = nc.alloc_sbuf_tensor("const-float32-neg4", [128, 1], FP)
    nc.gpsimd.memset(_c.ap(), -4.0)
    nc.const_aps.aps[(FP, -4.0)] = _c.ap()
    nc.all_engine_barrier()

    # [p, 2048*g + s] = hidden[s, 128*g + p]
    hT = nc.dram_tensor("hT", [128, 16 * S], F16, kind="ExternalInput").ap()
    # [p, 384*g + r] = wqkv_local[r, 128*g + p]; r: 0-255 q, 256-319 k, 320-383 v
    wq = nc.dram_tensor("wq", [128, 16 * 384], F16, kind="ExternalInput").ap()
    # [p, 2048*u + e] = wo[e, 256*core + 128*u + p]
    wo = nc.dram_tensor("wo", [128, 2 * 2048], F16, kind="ExternalInput").ap()
    # rope tables, full scale (the q weights carry 1/sqrt(hd)):
    # cols 0:2048 cos, 2048:4096 sin'
    rqq = nc.dram_tensor("rqq", [128, 2 * S], F16, kind="ExternalInput").ap()
    # cols 0:128 tri[p, f] = (p <= f); cols 128:256 identity[p, f] = (p == f)
    tri = nc.dram_tensor("tri", [128, 256], F16, kind="ExternalInput").ap()
    # f32 identity in rows 64-127 (rhs of the f32r v transposes)
    idf = nc.dram_tensor("idf", [128, 64], mybir.dt.float32r, kind="ExternalInput").ap()
    out = nc.dram_tensor("out", [S, D], F16, kind="ExternalOutput").ap()
    if debug:
        dbg = {
            name: nc.dram_tensor(f"dbg_{name}", shape, F16, kind="ExternalOutput").ap()
            for name, shape in (
                ("q01s", [128, S]),
                ("q23s", [128, S]),
                ("ks", [128, S]),
                ("v65", [128, 16 * 65]),
                ("o2a", [128, S]),
                ("o2b", [128, S]),
                ("ex0", [128, 8 * 1024]),
                ("sc0", [128, 8 * 1024]),
                ("po0", [65, 1024]),
                ("poc0", [65, 1024]),
                ("rbr0", [64, 1024]),
            )
        }

    with tile.TileContext(nc) as tc, ExitStack() as ctx:
        const = ctx.enter_context(tc.tile_pool(name="const", bufs=1))
        wqt = [
            const.tile([128, 4 * 384], F16, tag=f"wq{k}", name=f"wq_sb{k}")
            for k in range(4)
        ]

        def wq_ap(g, m):
            return wqt[g // 4][:, ds(384 * (g % 4) + 128 * m, 128)]

        F32R = mybir.dt.float32r
        wo_sb = const.tile([128, 4096], F16, tag="wo")
        rqq_sb = const.tile([128, 4096], F16, tag="rqq")
        tri_sb = const.tile([128, 256], F16, tag="tri")
        idf_sb = const.tile([128, 64], FP, tag="idf")
        ident64 = idf_sb[64:128, :].bitcast(F32R)

        qsw = ctx.enter_context(tc.tile_pool(name="qsw", bufs=1))
        q01s = qsw.tile([128, S], F16, tag="q01s")
        q23s = qsw.tile([128, S], F16, tag="q23s")
        ks = qsw.tile([128, S], F16, tag="ks")
        v_sb = qsw.tile([128, 16 * 65], F16, tag="v")
        o2a = qsw.tile([128, S], F16, tag="o2a")
        o2b = qsw.tile([128, S], F16, tag="o2b")
        v65 = v_sb.rearrange("p (j c) -> p j c", c=65)

        # attention pools (live through the whole kernel)
        expool = ctx.enter_context(tc.tile_pool(name="ex", bufs=4))
        rspool = ctx.enter_context(tc.tile_pool(name="rs", bufs=2))
        rcpool = ctx.enter_context(tc.tile_pool(name="rc", bufs=2))
        rbpool = ctx.enter_context(tc.tile_pool(name="rb", bufs=2))
        nmpool = ctx.enter_context(tc.tile_pool(name="nm", bufs=2))
        # attention psum pools are created after quarter 0 releases its
        # 3-bank ring (right side of the arena; release is LIFO per side)
        phaseA = ExitStack()
        psS = psO = None

        # projection-phase pools (close before the out-projection opens)
        phaseP = ExitStack()
        hpool = phaseP.enter_context(tc.tile_pool(name="hp", bufs=24))
        qraw = phaseP.enter_context(tc.tile_pool(name="qraw", bufs=1))
        q01 = qraw.tile([128, S], F16, tag="q01")
        q23 = qraw.tile([128, S], F16, tag="q23")
        kv = qraw.tile([128, S], F16, tag="kv")  # rows 0:64 = k (rope input)
        vraw = qraw.tile([128, S], FP, tag="vraw")  # rows 64:128 = v, f32
        scpool = phaseP.enter_context(tc.tile_pool(name="sc", bufs=4))
        phase0 = ExitStack()
        psA0 = phase0.enter_context(tc.tile_pool(name="psA0", bufs=1, space="PSUM"))
        psA = None

        SWAP_MASK = [i ^ 1 for i in range(32)]

        def rope_quarter(dst, raw, p, costab, sintab, q, nm):
            # dst = raw * cos + pairswap(raw) * sin' on [0:p, 512q:512q+512]
            cs = ds(512 * q, 512)
            sw = scpool.tile([128, 512], F16, tag="sc", name=f"sw_{nm}{q}")
            nc.vector.stream_shuffle(sw[0:p, :], raw[0:p, cs], SWAP_MASK)
            t0 = scpool.tile([128, 512], F16, tag="sc", name=f"t0_{nm}{q}")
            nc.vector.tensor_mul(t0[0:p, :], raw[0:p, cs], costab[0:p, cs])
            nc.vector.tensor_mul(sw[0:p, :], sw[0:p, :], sintab[0:p, cs])
            nc.vector.tensor_add(dst[0:p, cs], t0[0:p, :], sw[0:p, :])

        # global DMA plan: one ordered stream of large transfers.  hT comes as
        # 2048-column quads ([128, 4, 512] strided, >=1KB contiguous runs) so
        # each DMA is bus-bound, not HWDGE-dispatch-bound (625ns fixed cost).
        hT_r = hT.rearrange("p (g s) -> p g s", s=2048)
        rq_r = rqq.rearrange("p (h s) -> p h s", s=2048)
        hq_tiles = {}

        def dma_hq(q, t, half=None):
            # quad t of quarter q: g-chunks 4t..4t+3, columns 512q..512q+512
            if half is None:
                gs, n = 4 * t, 4
            else:
                gs, n = 4 * t + 2 * half, 2
            tile = hq_tiles.get((q, t))
            if tile is None:
                tile = hpool.tile([128, 4, 512], F16, tag="hc", name=f"hq_{q}_{t}")
                hq_tiles[(q, t)] = tile
            nc.sync.dma_start(
                tile[:, gs - 4 * t : gs - 4 * t + n, :],
                hT_r[:, gs : gs + n, ds(512 * q, 512)],
            )

        def hc_ap(q, g):
            return hq_tiles[(q, g // 4)][:, g % 4, :]

        rqsb_r = rqq_sb.rearrange("p (h s) -> p h s", s=2048)

        def dma_rq(q):
            nc.sync.dma_start(
                rqsb_r[:, :, ds(512 * q, 512)], rq_r[:, :, ds(512 * q, 512)]
            )

        dma_plan = [
            ("wq0a", lambda: nc.sync.dma_start(wqt[0][:, 0:768], wq[:, 0:768])),
            ("hq00a", lambda: dma_hq(0, 0, 0)),
            ("wq0b", lambda: nc.sync.dma_start(wqt[0][:, 768:1536], wq[:, 768:1536])),
            ("hq00b", lambda: dma_hq(0, 0, 1)),
            ("wq1", lambda: nc.sync.dma_start(wqt[1][:], wq[:, ds(1536, 1536)])),
            ("hq01", lambda: dma_hq(0, 1)),
            ("wq2", lambda: nc.sync.dma_start(wqt[2][:], wq[:, ds(3072, 1536)])),
            ("hq02", lambda: dma_hq(0, 2)),
            ("wq3", lambda: nc.sync.dma_start(wqt[3][:], wq[:, ds(4608, 1536)])),
            ("rq0", lambda: dma_rq(0)),
            ("tri", lambda: (nc.sync.dma_start(tri_sb[:], tri),
                             nc.sync.dma_start(idf_sb[:].bitcast(F32R), idf))),
            ("hq03", lambda: dma_hq(0, 3)),
            ("hq10", lambda: dma_hq(1, 0)),
            ("hq11", lambda: dma_hq(1, 1)),
            ("hq12", lambda: dma_hq(1, 2)),
            ("rq1", lambda: dma_rq(1)),
            ("hq13", lambda: dma_hq(1, 3)),
            ("hq20", lambda: dma_hq(2, 0)),
            ("wo0", lambda: nc.sync.dma_start(wo_sb[:, 0:2048], wo[:, 0:2048])),
            ("hq21", lambda: dma_hq(2, 1)),
            ("hq22", lambda: dma_hq(2, 2)),
            ("wo1", lambda: nc.sync.dma_start(wo_sb[:, 2048:4096], wo[:, 2048:4096])),
            ("rq2", lambda: dma_rq(2)),
            ("hq23", lambda: dma_hq(2, 3)),
            ("hq30", lambda: dma_hq(3, 0)),
            ("hq31", lambda: dma_hq(3, 1)),
            ("hq32", lambda: dma_hq(3, 2)),
            ("rq3", lambda: dma_rq(3)),
            ("hq33", lambda: dma_hq(3, 3)),
        ]
        plan_pos = {key: idx for idx, (key, _) in enumerate(dma_plan)}
        feed_state = {"next": 0}

        def feed_until(key):
            stop = plan_pos[key] + 1
            while feed_state["next"] < stop:
                dma_plan[feed_state["next"]][1]()
                feed_state["next"] += 1

        def emit_vtrans(q, vtp):
            # v transposes for quarter q: f32r through the projection psum
            # ring slot, then one f16 evacuation into v65
            vt16 = None
            vt = vtp.tile([128, 512], FP, tag=vtp_tag(vtp), name=f"vt_{q}")
            for jj in range(4):
                j = 4 * q + jj
                nc.tensor.transpose(
                    vt[:, ds(64 * jj, 64)].bitcast(F32R),
                    vraw[64:128, ds(128 * j, 128)].bitcast(F32R),
                    ident64,
                )
            nc.vector.tensor_copy(
                v65[:, 4 * q : 4 * q + 4, 0:64],
                vt[:, 0:256].rearrange("p (jj c) -> p jj c", c=64),
            )
            nc.vector.tensor_copy(
                v65[:, 4 * q : 4 * q + 4, 64:65],
                tri_sb[:, 127:128][:, None, :].to_broadcast([128, 4, 1]),
            )

        def vtp_tag(pool):
            return "p0" if pool is psA0 else "pj"

        def gen_proj_quarter0():
            # quarter 0 runs all three m-passes interleaved at the pace the
            # DMA stream can sustain (the front is inherently supply-bound)
            pts = [
                psA0.tile([128, 512], FP, tag=f"p{m}", name=f"pj0_{m}")
                for m in range(3)
            ]
            feed_until("hq00b")
            for g in range(12):
                feed_until(dma_plan[min(g + 4, 13)][0])
                for m in range(3):
                    nc.tensor.matmul(
                        pts[m][:], wq_ap(g, m), hc_ap(0, g),
                        start=(g == 0), stop=False,
                    )
                yield 639
            # finish the three passes m-serially, kv first: rope-k (which
            # gates the first attention scores) starts ~3us earlier
            cos_t, sin_t = rqq_sb[:, 0:2048], rqq_sb[:, 2048:4096]
            for m, dst, dsts, p, nm in (
                (2, kv, ks, 64, "k"),
                (0, q01, q01s, 128, "q01"),
                (1, q23, q23s, 128, "q23"),
            ):
                feed_until("hq11")
                for g in range(12, 16):
                    nc.tensor.matmul(
                        pts[m][:], wq_ap(g, m), hc_ap(0, g),
                        start=False, stop=(g == 15),
                    )
                if m == 2:
                    nc.scalar.copy(kv[0:64, 0:512], pts[m][0:64, :])
                    nc.scalar.copy(vraw[64:128, 0:512].bitcast(F32R), pts[m][64:128, :])
                else:
                    nc.scalar.copy(dst[:, 0:512], pts[m][:])
                rope_quarter(dsts, dst, p, cos_t, sin_t, 0, nm)
                if m == 2:
                    nc.sync.dma_start(ks[64:128, 0:512], ks[0:64, 0:512])
                yield 852
            emit_vtrans(0, psA0)
            yield 80
            feed_until("hq11")
            yield 120

        def gen_proj_quarter(q):
            feed_until(f"hq{q}1")
            pts = {}
            for m in (0, 1):
                pts[m] = psA.tile([128, 512], FP, tag="pj", name=f"pj_{q}_{m}")
            for g in range(16):
                if g % 4 == 2 and g < 12:
                    feed_until(f"hq{q}{g // 4 + 2}" if g // 4 + 2 <= 3 else f"hq{q}3")
                for m in (0, 1):
                    nc.tensor.matmul(
                        pts[m][:], wq_ap(g, m), hc_ap(q, g),
                        start=(g == 0), stop=(g == 15),
                    )
                yield 426
            cs = ds(512 * q, 512)
            nc.scalar.copy(q01[:, cs], pts[0][:])
            rope_quarter(q01s, q01, 128, rqq_sb[:, 0:2048], rqq_sb[:, 2048:4096], q, "q01")
            yield 300
            nc.scalar.copy(q23[:, cs], pts[1][:])
            rope_quarter(q23s, q23, 128, rqq_sb[:, 0:2048], rqq_sb[:, 2048:4096], q, "q23")
            yield 300
            pt2 = psA.tile([128, 512], FP, tag="pj", name=f"pj_{q}_2")
            for g in range(16):
                nc.tensor.matmul(
                    pt2[:], wq_ap(g, 2), hc_ap(q, g),
                    start=(g == 0), stop=(g == 15),
                )
                if g % 4 == 3:
                    # deep-prefetch the next quarter while the psum ring is
                    # the only DMA consumer
                    if q < 3:
                        feed_until(f"hq{q + 1}{min(g // 4, 3)}")
                    yield 852
            nc.scalar.copy(kv[0:64, cs], pt2[0:64, :])
            nc.scalar.copy(vraw[64:128, cs].bitcast(F32R), pt2[64:128, :])
            rope_quarter(ks, kv, 64, rqq_sb[:, 0:2048], rqq_sb[:, 2048:4096], q, "k")
            yield 300
            # duplicate rotated k at partitions 64-127 (odd heads' score
            # matmuls read lhsT/rhs both at base 64)
            nc.sync.dma_start(ks[64:128, cs], ks[0:64, cs])
            emit_vtrans(q, psA)
            if q < 3:
                feed_until(f"hq{q + 1}1")
            yield 120

        def gen_attention_chunk(c):
            nj = 4 * c + 4
            for hp in range(2):
                po = psO.tile([65, 1024], FP, tag="po", name=f"po_{c}_{hp}")

                def emit_scores(j):
                    r = j - 4 * c  # >= 0 on diagonal blocks
                    off = 128 * r if r >= 0 else 0
                    ps = psS.tile([128, 1024], FP, tag="ps", name=f"ps_{c}_{hp}_{j}")
                    for hh in range(2):
                        h = 2 * hp + hh
                        qt = q01s if h < 2 else q23s
                        base = 64 * (h % 2)
                        nc.tensor.matmul(
                            ps[:, ds(512 * hh + off, 512 - off)],
                            ks[base : base + 64, ds(128 * j, 128)],
                            qt[base : base + 64, ds(512 * c + off, 512 - off)],
                        )
                    return ps, off, r >= 0

                def emit_expav(j, ps, off, diag):
                    # exp(s - 4): softmax is shift-invariant and the bias
                    # keeps the unnormalized f16 sums (up to ~exp(9) * |v|)
                    # well inside f16 range
                    ex = expool.tile([128, 1024], F16, tag="ex", name=f"ex_{c}_{hp}_{j}")
                    if not diag:
                        nc.scalar.activation(ex[:], ps[:], EXP, bias=-4.0)
                    else:
                        w = 512 - off
                        psv = ps.rearrange("p (h w) -> p h w", w=512)[:, :, ds(off, w)]
                        exv = ex.rearrange("p (h w) -> p h w", w=512)[:, :, ds(off, w)]
                        nc.scalar.activation(exv, psv, EXP, bias=-4.0)
                        exd = ex.rearrange("p (h w) -> p h w", w=512)[:, :, ds(off, 128)]
                        nc.vector.tensor_mul(
                            exd,
                            exd,
                            tri_sb[:, 0:128][:, None, :].to_broadcast([128, 2, 128]),
                        )
                    if debug and c == 0:
                        sl = ds(1024 * (4 * hp + j), 1024)
                        nc.sync.dma_start(dbg["ex0"][:, sl], ex[:])
                        sc16 = expool.tile(
                            [128, 1024], F16, tag="ex", name=f"scd_{hp}_{j}"
                        )
                        nc.vector.tensor_copy(sc16[:], ps[:])
                        nc.sync.dma_start(dbg["sc0"][:, sl], sc16[:])
                    for hh in range(2):
                        nc.tensor.matmul(
                            po[0:65, ds(512 * hh + off, 512 - off)],
                            v_sb[:, ds(65 * j, 65)],
                            ex[:, ds(512 * hh + off, 512 - off)],
                            start=(j == 0),
                            stop=(j == nj - 1),
                            skip_group_check=True,
                        )

                # one-j lookahead: scores(j+1) land on the PE between
                # scores(j) and av(j) so the exp never stalls the PE
                def jcost(j):
                    # PE ns of one scores OR av pair at this block's trim
                    r = j - 4 * c
                    off = 128 * r if r >= 0 else 0
                    return int((512 - off) * 0.833)

                prev = emit_scores(0)
                for j in range(1, nj):
                    cur = emit_scores(j)
                    emit_expav(j - 1, *prev)
                    prev = cur
                    yield jcost(j) + jcost(j - 1)
                emit_expav(nj - 1, *prev)
                yield jcost(nj - 1)
                # evacuate the accumulator so the bank frees for the next
                # head pair, then normalize: reciprocal of the sums row,
                # partition_broadcast on the (idle) Pool engine, two muls
                poc = rspool.tile([65, 1024], F16, tag="rs", name=f"poc_{c}_{hp}")
                if debug and c == 0 and hp == 0:
                    pod = rspool.tile([65, 1024], F16, tag="pod", name="pod")
                    nc.vector.tensor_copy(pod[:], po[:])
                    nc.sync.dma_start(dbg["po0"], pod[:])
                nc.vector.tensor_copy(poc[:, 0:512], po[:, 0:512])
                nc.scalar.copy(poc[:, 512:1024], po[:, 512:1024])
                rbr = rbpool.tile([64, 1024], F16, tag="rbr", name=f"rbr_{c}_{hp}")
                dsttile = o2a if hp == 0 else o2b
                nm = nmpool.tile([64, 512], F16, tag="nm", name=f"nm_{c}_{hp}")
                # broadcast the sums row back into the (already-evacuated)
                # po bank with a ones-row matmul -- ones at base 64 to match
                # poc's denominator row -- then a lane-aligned reciprocal
                for half in range(2):
                    hs = ds(512 * half, 512)
                    nc.tensor.matmul(
                        po[0:64, hs], tri_sb[64:65, 64:128], poc[64:65, hs],
                        start=True, stop=True,
                    )
                    with nc.allow_low_precision(reason="softmax denom recip f16"):
                        nc.vector.reciprocal(rbr[0:64, hs], po[0:64, hs])
                    if half == 0:
                        nc.vector.tensor_mul(
                            dsttile[0:64, ds(512 * c, 512)],
                            poc[0:64, hs],
                            rbr[0:64, hs],
                        )
                    else:
                        nc.vector.tensor_mul(nm[0:64, :], poc[0:64, hs], rbr[0:64, hs])
                nc.sync.dma_start(dsttile[64:128, ds(512 * c, 512)], nm[0:64, :])
                if debug and c == 0 and hp == 0:
                    nc.sync.dma_start(dbg["poc0"], poc[:])
                    nc.sync.dma_start(dbg["rbr0"], rbr[0:64, :])
                yield 60

        post = {}

        def open_post_pools():
            post["ost"] = ctx.enter_context(tc.tile_pool(name="ost", bufs=6))
            post["psP"] = ctx.enter_context(tc.tile_pool(name="psP", bufs=2, space="PSUM"))

        def gen_outproj_chunk(c, tail=False, pskey="psP", bs=range(4)):
            for b in bs:
                for n2 in range(2):  # pairs of 512-wide e-slices -> one DMA
                    st = post["ost"].tile(
                        [128, 1024], F16, tag="st", name=f"st_{c}_{b}_{n2}"
                    )
                    for nn in range(2):
                        n = 2 * n2 + nn
                        pp = post[pskey].tile(
                            [128, 512], FP, tag="pp", name=f"pp_{c}_{b}_{n}"
                        )
                        nc.tensor.matmul(
                            pp[:],
                            o2a[:, ds(512 * c + 128 * b, 128)],
                            wo_sb[:, ds(512 * n, 512)],
                            start=True,
                            stop=False,
                        )
                        nc.tensor.matmul(
                            pp[:],
                            o2b[:, ds(512 * c + 128 * b, 128)],
                            wo_sb[:, ds(2048 + 512 * n, 512)],
                            start=False,
                            stop=True,
                        )
                        # in the pure-PE tail alternate evacuation engines so
                        # the psum ring keeps pace with the matmuls
                        if tail and nn == 1:
                            nc.scalar.copy(st[:, ds(512, 512)], pp[:])
                        else:
                            nc.vector.tensor_copy(st[:, ds(512 * nn, 512)], pp[:])
                        yield 426
                    nc.sync.dma_start(
                        out[ds(128 * (4 * c + b), 128), ds(1024 * n2, 1024)], st[:]
                    )

        def chain(*gens):
            for g in gens:
                yield from g

        def closer():
            phaseP.close()
            open_post_pools()
            return
            yield  # pragma: no cover

        def weave(ga, gb, wa=1.0, wb=1.0):
            # proportional-progress interleave of two emission streams:
            # step the stream with the smaller fraction-complete so a short
            # filler spreads across the whole window instead of front-loading
            ta = tb = 0.0
            da = db = False
            while not (da and db):
                if db or (not da and ta / wa <= tb / wb):
                    try:
                        ta += next(ga)
                    except StopIteration:
                        da = True
                else:
                    try:
                        tb += next(gb)
                    except StopIteration:
                        db = True

        def run(g):
            for _ in g:
                pass

        # ---- pipeline: P0 [P1|A0] [P2|A1] [P3,close,O0|A2] [O1,O2|A3] O3 --
        run(gen_proj_quarter0())
        phase0.close()
        psA = phaseP.enter_context(tc.tile_pool(name="psA", bufs=2, space="PSUM"))
        psS = phaseA.enter_context(
            tc.tile_pool(name="psS", bufs=2, space="PSUM", side="right")
        )
        psO = phaseA.enter_context(
            tc.tile_pool(name="psO", bufs=1, space="PSUM", side="right")
        )
        weave(gen_proj_quarter(1), gen_attention_chunk(0))
        weave(gen_proj_quarter(2), gen_attention_chunk(1))
        weave(
            chain(gen_proj_quarter(3), closer(), gen_outproj_chunk(0)),
            gen_attention_chunk(2),
            wa=17.5,
            wb=16.6,
        )
        weave(
            chain(gen_outproj_chunk(1), gen_outproj_chunk(2, bs=range(3))),
            gen_attention_chunk(3),
            wa=11.9,
            wb=22.1,
        )
        # attention psum freed -> deep out-proj ring; the O2 remainder hides
        # the last normalize chain before O3 starts
        phaseA.close()
        post["psP2"] = ctx.enter_context(
            tc.tile_pool(name="psP2", bufs=4, space="PSUM", side="right")
        )
        run(gen_outproj_chunk(2, tail=True, pskey="psP2", bs=range(3, 4)))
        run(gen_outproj_chunk(3, tail=True, pskey="psP2"))
        if debug:
            for name, tile in (
                ("q01s", q01s), ("q23s", q23s), ("ks", ks),
                ("v65", v_sb), ("o2a", o2a), ("o2b", o2b),
            ):
                nc.sync.dma_start(dbg[name], tile[:])

    nc.compile()
    return nc


def get_module(debug=False):
    key = ("nc", debug)
    if key not in _CACHE:
        _CACHE[key] = _build_module(debug=debug)
    return _CACHE[key]


def _pack16(x):
    # [16*128, N] -> [128, 16*N] with [p, N*g + n] = x[128*g + p, n]
    n = x.shape[1]
    return (
        np.ascontiguousarray(x.reshape(16, 128, n).transpose(1, 0, 2)).reshape(128, 16 * n)
    )


def prep_inputs(hidden_states, freqs_cis, wqkv, wo):
    h = np.asarray(hidden_states, dtype=np.float32)[0]  # [S, D]
    fc = np.asarray(freqs_cis, dtype=np.float32)  # [S, 32, 2]
    wqkv = np.asarray(wqkv, dtype=np.float32)  # [3072, D]
    wo = np.asarray(wo, dtype=np.float32)  # [D, D]

    hT_sb = _pack16(np.ascontiguousarray(h.T)).astype(np.float16)

    cos = fc[:, :, 0]  # [S, 32]
    sin = fc[:, :, 1]
    cos_ext = np.repeat(cos, 2, axis=1).T  # [64, S]
    sgn = np.where(np.arange(HD) % 2 == 0, -1.0, 1.0).astype(np.float32)[:, None]
    sin_ext = np.repeat(sin, 2, axis=1).T * sgn  # sin'[d, s]
    rqq_np = np.concatenate(
        [np.tile(cos_ext, (2, 1)), np.tile(sin_ext, (2, 1))], axis=1
    ).astype(np.float16)  # [128, 4096] full scale
    idf_np = np.zeros((128, 64), dtype=np.float32)
    idf_np[64:128] = np.eye(64, dtype=np.float32)
    tri_np = np.concatenate(
        [
            (np.arange(128)[:, None] <= np.arange(128)[None, :]).astype(np.float16),
            np.eye(128, dtype=np.float16),
        ],
        axis=1,
    )  # [128, 256]: triangle | identity

    in_maps = []
    for i in range(NCORES):
        scale = 1.0 / np.sqrt(np.float32(HD))
        wl = np.concatenate(
            [
                wqkv[256 * i : 256 * i + 256] * scale,
                wqkv[D + 64 * i : D + 64 * i + 64],
                wqkv[D + KV_SIZE + 64 * i : D + KV_SIZE + 64 * i + 64],
            ],
            axis=0,
        )  # [384, D]
        wq_sb = _pack16(np.ascontiguousarray(wl.T)).astype(np.float16)
        woT = np.ascontiguousarray(wo[:, 256 * i : 256 * i + 256].T)  # [256, D]
        wo_sb = (
            np.ascontiguousarray(woT.reshape(2, 128, D).transpose(1, 0, 2))
            .reshape(128, 2 * D)
            .astype(np.float16)
        )
        in_maps.append(
            {
                "hT": hT_sb,
                "wq": wq_sb,
                "wo": wo_sb,
                "rqq": rqq_np,
                "tri": tri_np,
                "idf": idf_np,
            }
        )
    return in_maps


def run_on_hw(in_maps, trace=False, **kw):
    from concourse.bass_utils import run_bass_kernel_spmd

    nc = get_module()
    return run_bass_kernel_spmd(nc, in_maps, list(range(NCORES)), trace=trace, **kw)


def kernel(hidden_states, freqs_cis, wqkv, wo):
    in_maps = prep_inputs(hidden_states, freqs_cis, wqkv, wo)
    res = run_on_hw(in_maps)
    acc = np.zeros((S, D), dtype=np.float64)
    for r in res.results:
        acc += np.asarray(r["out"], dtype=np.float64)
    return acc.astype(np.float32).reshape(1, S, D)


# revision 54
# speedup vs baseline: 1.0162x; 1.0084x over previous
"""Tensor-parallel (over GQA head groups) multi-head attention for 8 trn2 cores.

Each core owns 4 query heads + their shared kv head (one GQA group), the
matching 384 rows of wqkv and 256 columns of wo.  Every core computes a full
[S, D] partial of the output projection; the host sums the 8 partials.

v2: fp16 data end-to-end (host converts; PSUM stays fp32) and a software-
pipelined schedule that keeps the PE busy continuously:
  - projection runs in four 512-column quarters, each as three m-serial
    passes (q01/q23/kv) over 16 resident hT chunks -> only 2 PSUM banks,
    so projection overlaps attention in PSUM;
  - attention chunk c is emitted between projection quarters c+1/c+2;
    out-projection chunks are emitted after the projection pools close;
  - scores on diagonal blocks restrict the moving operand to the live
    triangle columns (128-granular staircase);
  - the softmax denominator broadcast reuses the evacuated po psum bank
    (ones-row matmul per bank, then a lane-aligned reciprocal);
  - exp is computed as exp(s - 4) so the unnormalized f16 sums stay in
    range (softmax is shift-invariant);
  - v is kept in f32 (second half of the kv evacuation) and transposed
    f32r through the projection psum ring.
Dataflow inside one core otherwise as v1 (scores transposed [ks, qs], ones
column in V for the denominator).
"""

import sys

if "/opt/trn_rl_repo" not in sys.path:
    sys.path.insert(0, "/opt/trn_rl_repo")

import numpy as np

S = 2048
D = 2048
HD = 64
N_HEAD = 32
N_KV = 8
NCORES = 8
QH_PER_CORE = N_HEAD // NCORES  # 4
KV_SIZE = N_KV * HD  # 512

_CACHE = {}


def _build_module(debug=False):
    from contextlib import ExitStack

    import concourse.mybir as mybir
    import concourse.tile as tile
    from concourse import bacc
    from concourse.bass import ds

    FP = mybir.dt.float32
    F16 = mybir.dt.float16
    EXP = mybir.ActivationFunctionType.Exp

    nc = bacc.Bacc(
        "TRN2",
        target_bir_lowering=False,
        debug=False,
        enable_asserts=False,
        num_devices=NCORES,
    )
    # register the exp bias constant (softmax shift, see emit_expav)
    _c = nc.alloc_sbuf_tensor("const-float32-neg4", [128, 1], FP)
    nc.gpsimd.memset(_c.ap(), -4.0)
    nc.const_aps.aps[(FP, -4.0)] = _c.ap()
    nc.all_engine_barrier()

    # [p, 2048*g + s] = hidden[s, 128*g + p]
    hT = nc.dram_tensor("hT", [128, 16 * S], F16, kind="ExternalInput").ap()
    # [p, 384*g + r] = wqkv_local[r, 128*g + p]; r: 0-255 q, 256-319 k, 320-383 v
    wq = nc.dram_tensor("wq", [128, 16 * 384], F16, kind="ExternalInput").ap()
    # [p, 2048*u + e] = wo[e, 256*core + 128*u + p]
    wo = nc.dram_tensor("wo", [128, 2 * 2048], F16, kind="ExternalInput").ap()
    # rope tables, full scale (the q weights carry 1/sqrt(hd)):
    # cols 0:2048 cos, 2048:4096 sin'
    rqq = nc.dram_tensor("rqq", [128, 2 * S], F16, kind="ExternalInput").ap()
    # cols 0:128 tri[p, f] = (p <= f); cols 128:256 identity[p, f] = (p == f)
    tri = nc.dram_tensor("tri", [128, 256], F16, kind="ExternalInput").ap()
    # f32 identity in rows 64-127 (rhs of the f32r v transposes)
    idf = nc.dram_tensor("idf", [128, 64], mybir.dt.float32r, kind="ExternalInput").ap()
    out = nc.dram_tensor("out", [S, D], F16, kind="ExternalOutput").ap()
    if debug:
        dbg = {
            name: nc.dram_tensor(f"dbg_{name}", shape, F16, kind="ExternalOutput").ap()
            for name, shape in (
                ("q01s", [128, S]),
                ("q23s", [128, S]),
                ("ks", [128, S]),
                ("v65", [128, 16 * 65]),
                ("o2a", [128, S]),
                ("o2b", [128, S]),
                ("ex0", [128, 8 * 1024]),
                ("sc0", [128, 8 * 1024]),
                ("po0", [65, 1024]),
                ("poc0", [65, 1024]),
                ("rbr0", [64, 1024]),
            )
        }

    with tile.TileContext(nc) as tc, ExitStack() as ctx:
        const = ctx.enter_context(tc.tile_pool(name="const", bufs=1))
        wqt = [
            const.tile([128, 4 * 384], F16, tag=f"wq{k}", name=f"wq_sb{k}")
            for k in range(4)
        ]

        def wq_ap(g, m):
            return wqt[g // 4][:, ds(384 * (g % 4) + 128 * m, 128)]

        F32R = mybir.dt.float32r
        wo_sb = const.tile([128, 4096], F16, tag="wo")
        rqq_sb = const.tile([128, 4096], F16, tag="rqq")
        tri_sb = const.tile([128, 256], F16, tag="tri")
        idf_sb = const.tile([128, 64], FP, tag="idf")
        ident64 = idf_sb[64:128, :].bitcast(F32R)

        qsw = ctx.enter_context(tc.tile_pool(name="qsw", bufs=1))
        q01s = qsw.tile([128, S], F16, tag="q01s")
        q23s = qsw.tile([128, S], F16, tag="q23s")
        ks = qsw.tile([128, S], F16, tag="ks")
        v_sb = qsw.tile([128, 16 * 65], F16, tag="v")
        o2a = qsw.tile([128, S], F16, tag="o2a")
        o2b = qsw.tile([128, S], F16, tag="o2b")
        v65 = v_sb.rearrange("p (j c) -> p j c", c=65)

        # attention pools (live through the whole kernel)
        expool = ctx.enter_context(tc.tile_pool(name="ex", bufs=4))
        rspool = ctx.enter_context(tc.tile_pool(name="rs", bufs=2))
        rcpool = ctx.enter_context(tc.tile_pool(name="rc", bufs=2))
        rbpool = ctx.enter_context(tc.tile_pool(name="rb", bufs=2))
        nmpool = ctx.enter_context(tc.tile_pool(name="nm", bufs=2))
        # attention psum pools are created after quarter 0 releases its
        # 3-bank ring (right side of the arena; release is LIFO per side)
        phaseA = ExitStack()
        psS = psO = None

        # projection-phase pools (close before the out-projection opens)
        phaseP = ExitStack()
        hpool = phaseP.enter_context(tc.tile_pool(name="hp", bufs=24))
        qraw = phaseP.enter_context(tc.tile_pool(name="qraw", bufs=1))
        q01 = qraw.tile([128, S], F16, tag="q01")
        q23 = qraw.tile([128, S], F16, tag="q23")
        kv = qraw.tile([128, S], F16, tag="kv")  # rows 0:64 = k (rope input)
        vraw = qraw.tile([128, S], FP, tag="vraw")  # rows 64:128 = v, f32
        scpool = phaseP.enter_context(tc.tile_pool(name="sc", bufs=4))
        phase0 = ExitStack()
        psA0 = phase0.enter_context(tc.tile_pool(name="psA0", bufs=1, space="PSUM"))
        psA = None

        SWAP_MASK = [i ^ 1 for i in range(32)]

        def rope_quarter(dst, raw, p, costab, sintab, q, nm):
            # dst = raw * cos + pairswap(raw) * sin' on [0:p, 512q:512q+512]
            cs = ds(512 * q, 512)
            sw = scpool.tile([128, 512], F16, tag="sc", name=f"sw_{nm}{q}")
            nc.vector.stream_shuffle(sw[0:p, :], raw[0:p, cs], SWAP_MASK)
            t0 = scpool.tile([128, 512], F16, tag="sc", name=f"t0_{nm}{q}")
            nc.vector.tensor_mul(t0[0:p, :], raw[0:p, cs], costab[0:p, cs])
            nc.vector.tensor_mul(sw[0:p, :], sw[0:p, :], sintab[0:p, cs])
            nc.vector.tensor_add(dst[0:p, cs], t0[0:p, :], sw[0:p, :])

        # global DMA plan: one ordered stream of large transfers.  hT comes as
        # 2048-column quads ([128, 4, 512] strided, >=1KB contiguous runs) so
        # each DMA is bus-bound, not HWDGE-dispatch-bound (625ns fixed cost).
        hT_r = hT.rearrange("p (g s) -> p g s", s=2048)
        rq_r = rqq.rearrange("p (h s) -> p h s", s=2048)
        hq_tiles = {}

        def dma_hq(q, t, half=None):
            # quad t of quarter q: g-chunks 4t..4t+3, columns 512q..512q+512
            if half is None:
                gs, n = 4 * t, 4
            else:
                gs, n = 4 * t + 2 * half, 2
            tile = hq_tiles.get((q, t))
            if tile is None:
                tile = hpool.tile([128, 4, 512], F16, tag="hc", name=f"hq_{q}_{t}")
                hq_tiles[(q, t)] = tile
            nc.sync.dma_start(
                tile[:, gs - 4 * t : gs - 4 * t + n, :],
                hT_r[:, gs : gs + n, ds(512 * q, 512)],
            )

        def hc_ap(q, g):
            return hq_tiles[(q, g // 4)][:, g % 4, :]

        rqsb_r = rqq_sb.rearrange("p (h s) -> p h s", s=2048)

        def dma_rq(q):
            nc.sync.dma_start(
                rqsb_r[:, :, ds(512 * q, 512)], rq_r[:, :, ds(512 * q, 512)]
            )

        dma_plan = [
            ("wq0a", lambda: nc.sync.dma_start(wqt[0][:, 0:768], wq[:, 0:768])),
            ("hq00a", lambda: dma_hq(0, 0, 0)),
            ("wq0b", lambda: nc.sync.dma_start(wqt[0][:, 768:1536], wq[:, 768:1536])),
            ("hq00b", lambda: dma_hq(0, 0, 1)),
            ("wq1", lambda: nc.sync.dma_start(wqt[1][:], wq[:, ds(1536, 1536)])),
            ("hq01", lambda: dma_hq(0, 1)),
            ("wq2", lambda: nc.sync.dma_start(wqt[2][:], wq[:, ds(3072, 1536)])),
            ("hq02", lambda: dma_hq(0, 2)),
            ("wq3", lambda: nc.sync.dma_start(wqt[3][:], wq[:, ds(4608, 1536)])),
            ("hq03", lambda: dma_hq(0, 3)),
            ("rq0", lambda: dma_rq(0)),
            ("tri", lambda: (nc.sync.dma_start(tri_sb[:], tri),
                             nc.sync.dma_start(idf_sb[:].bitcast(F32R), idf))),
            ("hq10", lambda: dma_hq(1, 0)),
            ("hq11", lambda: dma_hq(1, 1)),
            ("hq12", lambda: dma_hq(1, 2)),
            ("rq1", lambda: dma_rq(1)),
            ("hq13", lambda: dma_hq(1, 3)),
            ("hq20", lambda: dma_hq(2, 0)),
            ("wo0", lambda: nc.sync.dma_start(wo_sb[:, 0:2048], wo[:, 0:2048])),
            ("hq21", lambda: dma_hq(2, 1)),
            ("hq22", lambda: dma_hq(2, 2)),
            ("wo1", lambda: nc.sync.dma_start(wo_sb[:, 2048:4096], wo[:, 2048:4096])),
            ("rq2", lambda: dma_rq(2)),
            ("hq23", lambda: dma_hq(2, 3)),
            ("hq30", lambda: dma_hq(3, 0)),
            ("hq31", lambda: dma_hq(3, 1)),
            ("hq32", lambda: dma_hq(3, 2)),
            ("rq3", lambda: dma_rq(3)),
            ("hq33", lambda: dma_hq(3, 3)),
        ]
        plan_pos = {key: idx for idx, (key, _) in enumerate(dma_plan)}
        feed_state = {"next": 0}

        def feed_until(key):
            stop = plan_pos[key] + 1
            while feed_state["next"] < stop:
                dma_plan[feed_state["next"]][1]()
                feed_state["next"] += 1

        def emit_vtrans(q, vtp):
            # v transposes for quarter q: f32r through the projection psum
            # ring slot, then one f16 evacuation into v65
            vt16 = None
            vt = vtp.tile([128, 512], FP, tag=vtp_tag(vtp), name=f"vt_{q}")
            for jj in range(4):
                j = 4 * q + jj
                nc.tensor.transpose(
                    vt[:, ds(64 * jj, 64)].bitcast(F32R),
                    vraw[64:128, ds(128 * j, 128)].bitcast(F32R),
                    ident64,
                )
            nc.vector.tensor_copy(
                v65[:, 4 * q : 4 * q + 4, 0:64],
                vt[:, 0:256].rearrange("p (jj c) -> p jj c", c=64),
            )
            nc.vector.tensor_copy(
                v65[:, 4 * q : 4 * q + 4, 64:65],
                tri_sb[:, 127:128][:, None, :].to_broadcast([128, 4, 1]),
            )

        def vtp_tag(pool):
            return "p0" if pool is psA0 else "pj"

        def gen_proj_quarter0():
            # quarter 0 runs all three m-passes interleaved at the pace the
            # DMA stream can sustain (the front is inherently supply-bound)
            pts = [
                psA0.tile([128, 512], FP, tag=f"p{m}", name=f"pj0_{m}")
                for m in range(3)
            ]
            feed_until("hq00b")
            for g in range(12):
                feed_until(dma_plan[min(g + 4, 13)][0])
                for m in range(3):
                    nc.tensor.matmul(
                        pts[m][:], wq_ap(g, m), hc_ap(0, g),
                        start=(g == 0), stop=False,
                    )
                yield 639
            # finish the three passes m-serially, kv first: rope-k (which
            # gates the first attention scores) starts ~3us earlier
            cos_t, sin_t = rqq_sb[:, 0:2048], rqq_sb[:, 2048:4096]
            for m, dst, dsts, p, nm in (
                (2, kv, ks, 64, "k"),
                (0, q01, q01s, 128, "q01"),
                (1, q23, q23s, 128, "q23"),
            ):
                feed_until("hq11")
                for g in range(12, 16):
                    nc.tensor.matmul(
                        pts[m][:], wq_ap(g, m), hc_ap(0, g),
                        start=False, stop=(g == 15),
                    )
                if m == 2:
                    nc.scalar.copy(kv[0:64, 0:512], pts[m][0:64, :])
                    nc.scalar.copy(vraw[64:128, 0:512].bitcast(F32R), pts[m][64:128, :])
                else:
                    nc.scalar.copy(dst[:, 0:512], pts[m][:])
                rope_quarter(dsts, dst, p, cos_t, sin_t, 0, nm)
                if m == 2:
                    nc.sync.dma_start(ks[64:128, 0:512], ks[0:64, 0:512])
                yield 852
            emit_vtrans(0, psA0)
            yield 80
            feed_until("hq11")
            yield 120

        def gen_proj_quarter(q):
            feed_until(f"hq{q}1")
            pts = {}
            for m in (0, 1):
                pts[m] = psA.tile([128, 512], FP, tag="pj", name=f"pj_{q}_{m}")
            for g in range(16):
                if g % 4 == 2 and g < 12:
                    feed_until(f"hq{q}{g // 4 + 2}" if g // 4 + 2 <= 3 else f"hq{q}3")
                for m in (0, 1):
                    nc.tensor.matmul(
                        pts[m][:], wq_ap(g, m), hc_ap(q, g),
                        start=(g == 0), stop=(g == 15),
                    )
                yield 426
            cs = ds(512 * q, 512)
            nc.scalar.copy(q01[:, cs], pts[0][:])
            rope_quarter(q01s, q01, 128, rqq_sb[:, 0:2048], rqq_sb[:, 2048:4096], q, "q01")
            yield 300
            nc.scalar.copy(q23[:, cs], pts[1][:])
            rope_quarter(q23s, q23, 128, rqq_sb[:, 0:2048], rqq_sb[:, 2048:4096], q, "q23")
            yield 300
            pt2 = psA.tile([128, 512], FP, tag="pj", name=f"pj_{q}_2")
            for g in range(16):
                nc.tensor.matmul(
                    pt2[:], wq_ap(g, 2), hc_ap(q, g),
                    start=(g == 0), stop=(g == 15),
                )
                if g % 4 == 3:
                    # deep-prefetch the next quarter while the psum ring is
                    # the only DMA consumer
                    if q < 3:
                        feed_until(f"hq{q + 1}{min(g // 4, 3)}")
                    yield 852
            nc.scalar.copy(kv[0:64, cs], pt2[0:64, :])
            nc.scalar.copy(vraw[64:128, cs].bitcast(F32R), pt2[64:128, :])
            rope_quarter(ks, kv, 64, rqq_sb[:, 0:2048], rqq_sb[:, 2048:4096], q, "k")
            yield 300
            # duplicate rotated k at partitions 64-127 (odd heads' score
            # matmuls read lhsT/rhs both at base 64)
            nc.sync.dma_start(ks[64:128, cs], ks[0:64, cs])
            emit_vtrans(q, psA)
            if q < 3:
                feed_until(f"hq{q + 1}1")
            yield 120

        def gen_attention_chunk(c):
            nj = 4 * c + 4
            for hp in range(2):
                po = psO.tile([65, 1024], FP, tag="po", name=f"po_{c}_{hp}")

                def emit_scores(j):
                    r = j - 4 * c  # >= 0 on diagonal blocks
                    off = 128 * r if r >= 0 else 0
                    ps = psS.tile([128, 1024], FP, tag="ps", name=f"ps_{c}_{hp}_{j}")
                    for hh in range(2):
                        h = 2 * hp + hh
                        qt = q01s if h < 2 else q23s
                        base = 64 * (h % 2)
                        nc.tensor.matmul(
                            ps[:, ds(512 * hh + off, 512 - off)],
                            ks[base : base + 64, ds(128 * j, 128)],
                            qt[base : base + 64, ds(512 * c + off, 512 - off)],
                        )
                    return ps, off, r >= 0

                def emit_expav(j, ps, off, diag):
                    # exp(s - 4): softmax is shift-invariant and the bias
                    # keeps the unnormalized f16 sums (up to ~exp(9) * |v|)
                    # well inside f16 range
                    ex = expool.tile([128, 1024], F16, tag="ex", name=f"ex_{c}_{hp}_{j}")
                    if not diag:
                        nc.scalar.activation(ex[:], ps[:], EXP, bias=-4.0)
                    else:
                        w = 512 - off
                        psv = ps.rearrange("p (h w) -> p h w", w=512)[:, :, ds(off, w)]
                        exv = ex.rearrange("p (h w) -> p h w", w=512)[:, :, ds(off, w)]
                        nc.scalar.activation(exv, psv, EXP, bias=-4.0)
                        exd = ex.rearrange("p (h w) -> p h w", w=512)[:, :, ds(off, 128)]
                        nc.vector.tensor_mul(
                            exd,
                            exd,
                            tri_sb[:, 0:128][:, None, :].to_broadcast([128, 2, 128]),
                        )
                    if debug and c == 0:
                        sl = ds(1024 * (4 * hp + j), 1024)
                        nc.sync.dma_start(dbg["ex0"][:, sl], ex[:])
                        sc16 = expool.tile(
                            [128, 1024], F16, tag="ex", name=f"scd_{hp}_{j}"
                        )
                        nc.vector.tensor_copy(sc16[:], ps[:])
                        nc.sync.dma_start(dbg["sc0"][:, sl], sc16[:])
                    for hh in range(2):
                        nc.tensor.matmul(
                            po[0:65, ds(512 * hh + off, 512 - off)],
                            v_sb[:, ds(65 * j, 65)],
                            ex[:, ds(512 * hh + off, 512 - off)],
                            start=(j == 0),
                            stop=(j == nj - 1),
                            skip_group_check=True,
                        )

                # one-j lookahead: scores(j+1) land on the PE between
                # scores(j) and av(j) so the exp never stalls the PE
                def jcost(j):
                    # PE ns of one scores OR av pair at this block's trim
                    r = j - 4 * c
                    off = 128 * r if r >= 0 else 0
                    return int((512 - off) * 0.833)

                prev = emit_scores(0)
                for j in range(1, nj):
                    cur = emit_scores(j)
                    emit_expav(j - 1, *prev)
                    prev = cur
                    yield jcost(j) + jcost(j - 1)
                emit_expav(nj - 1, *prev)
                yield jcost(nj - 1)
                # evacuate the accumulator so the bank frees for the next
                # head pair, then normalize: reciprocal of the sums row,
                # partition_broadcast on the (idle) Pool engine, two muls
                poc = rspool.tile([65, 1024], F16, tag="rs", name=f"poc_{c}_{hp}")
                if debug and c == 0 and hp == 0:
                    pod = rspool.tile([65, 1024], F16, tag="pod", name="pod")
                    nc.vector.tensor_copy(pod[:], po[:])
                    nc.sync.dma_start(dbg["po0"], pod[:])
                nc.vector.tensor_copy(poc[:, 0:512], po[:, 0:512])
                nc.scalar.copy(poc[:, 512:1024], po[:, 512:1024])
                rbr = rbpool.tile([64, 1024], F16, tag="rbr", name=f"rbr_{c}_{hp}")
                dsttile = o2a if hp == 0 else o2b
                nm = nmpool.tile([64, 512], F16, tag="nm", name=f"nm_{c}_{hp}")
                # broadcast the sums row back into the (already-evacuated)
                # po bank with a ones-row matmul -- ones at base 64 to match
                # poc's denominator row -- then a lane-aligned reciprocal
                for half in range(2):
                    hs = ds(512 * half, 512)
                    nc.tensor.matmul(
                        po[0:64, hs], tri_sb[64:65, 64:128], poc[64:65, hs],
                        start=True, stop=True,
                    )
                    with nc.allow_low_precision(reason="softmax denom recip f16"):
                        nc.vector.reciprocal(rbr[0:64, hs], po[0:64, hs])
                    if half == 0:
                        nc.vector.tensor_mul(
                            dsttile[0:64, ds(512 * c, 512)],
                            poc[0:64, hs],
                            rbr[0:64, hs],
                        )
                    else:
                        nc.vector.tensor_mul(nm[0:64, :], poc[0:64, hs], rbr[0:64, hs])
                nc.sync.dma_start(dsttile[64:128, ds(512 * c, 512)], nm[0:64, :])
                if debug and c == 0 and hp == 0:
                    nc.sync.dma_start(dbg["poc0"], poc[:])
                    nc.sync.dma_start(dbg["rbr0"], rbr[0:64, :])
                yield 60

        post = {}

        def open_post_pools():
            post["ost"] = ctx.enter_context(tc.tile_pool(name="ost", bufs=6))
            post["psP"] = ctx.enter_context(tc.tile_pool(name="psP", bufs=2, space="PSUM"))

        def gen_outproj_chunk(c, tail=False, pskey="psP", bs=range(4)):
            for b in bs:
                for n2 in range(2):  # pairs of 512-wide e-slices -> one DMA
                    st = post["ost"].tile(
                        [128, 1024], F16, tag="st", name=f"st_{c}_{b}_{n2}"
                    )
                    for nn in range(2):
                        n = 2 * n2 + nn
                        pp = post[pskey].tile(
                            [128, 512], FP, tag="pp", name=f"pp_{c}_{b}_{n}"
                        )
                        nc.tensor.matmul(
                            pp[:],
                            o2a[:, ds(512 * c + 128 * b, 128)],
                            wo_sb[:, ds(512 * n, 512)],
                            start=True,
                            stop=False,
                        )
                        nc.tensor.matmul(
                            pp[:],
                            o2b[:, ds(512 * c + 128 * b, 128)],
                            wo_sb[:, ds(2048 + 512 * n, 512)],
                            start=False,
                            stop=True,
                        )
                        # in the pure-PE tail alternate evacuation engines so
                        # the psum ring keeps pace with the matmuls
                        if tail and nn == 1:
                            nc.scalar.copy(st[:, ds(512, 512)], pp[:])
                        else:
                            nc.vector.tensor_copy(st[:, ds(512 * nn, 512)], pp[:])
                        yield 426
                    nc.sync.dma_start(
                        out[ds(128 * (4 * c + b), 128), ds(1024 * n2, 1024)], st[:]
                    )

        def chain(*gens):
            for g in gens:
                yield from g

        def closer():
            phaseP.close()
            open_post_pools()
            return
            yield  # pragma: no cover

        def weave(ga, gb, wa=1.0, wb=1.0):
            # proportional-progress interleave of two emission streams:
            # step the stream with the smaller fraction-complete so a short
            # filler spreads across the whole window instead of front-loading
            ta = tb = 0.0
            da = db = False
            while not (da and db):
                if db or (not da and ta / wa <= tb / wb):
                    try:
                        ta += next(ga)
                    except StopIteration:
                        da = True
                else:
                    try:
                        tb += next(gb)
                    except StopIteration:
                        db = True

        def run(g):
            for _ in g:
                pass

        # ---- pipeline: P0 [P1|A0] [P2|A1] [P3,close,O0|A2] [O1,O2|A3] O3 --
        run(gen_proj_quarter0())
        phase0.close()
        psA = phaseP.enter_context(tc.tile_pool(name="psA", bufs=2, space="PSUM"))
        psS = phaseA.enter_context(
            tc.tile_pool(name="psS", bufs=2, space="PSUM", side="right")
        )
        psO = phaseA.enter_context(
            tc.tile_pool(name="psO", bufs=1, space="PSUM", side="right")
        )
        weave(gen_proj_quarter(1), gen_attention_chunk(0))
        weave(gen_proj_quarter(2), gen_attention_chunk(1))
        weave(
            chain(gen_proj_quarter(3), closer(), gen_outproj_chunk(0)),
            gen_attention_chunk(2),
            wa=17.5,
            wb=16.6,
        )
        weave(
            chain(gen_outproj_chunk(1), gen_outproj_chunk(2, bs=range(3))),
            gen_attention_chunk(3),
            wa=11.9,
            wb=22.1,
        )
        # attention psum freed -> deep out-proj ring; the O2 remainder hides
        # the last normalize chain before O3 starts
        phaseA.close()
        post["psP2"] = ctx.enter_context(
            tc.tile_pool(name="psP2", bufs=4, space="PSUM", side="right")
        )
        run(gen_outproj_chunk(2, tail=True, pskey="psP2", bs=range(3, 4)))
        run(gen_outproj_chunk(3, tail=True, pskey="psP2"))
        if debug:
            for name, tile in (
                ("q01s", q01s), ("q23s", q23s), ("ks", ks),
                ("v65", v_sb), ("o2a", o2a), ("o2b", o2b),
            ):
                nc.sync.dma_start(dbg[name], tile[:])

    nc.compile()
    return nc


def get_module(debug=False):
    key = ("nc", debug)
    if key not in _CACHE:
        _CACHE[key] = _build_module(debug=debug)
    return _CACHE[key]


def _pack16(x):
    # [16*128, N] -> [128, 16*N] with [p, N*g + n] = x[128*g + p, n]
    n = x.shape[1]
    return (
        np.ascontiguousarray(x.reshape(16, 128, n).transpose(1, 0, 2)).reshape(128, 16 * n)
    )


def prep_inputs(hidden_states, freqs_cis, wqkv, wo):
    h = np.asarray(hidden_states, dtype=np.float32)[0]  # [S, D]
    fc = np.asarray(freqs_cis, dtype=np.float32)  # [S, 32, 2]
    wqkv = np.asarray(wqkv, dtype=np.float32)  # [3072, D]
    wo = np.asarray(wo, dtype=np.float32)  # [D, D]

    hT_sb = _pack16(np.ascontiguousarray(h.T)).astype(np.float16)

    cos = fc[:, :, 0]  # [S, 32]
    sin = fc[:, :, 1]
    cos_ext = np.repeat(cos, 2, axis=1).T  # [64, S]
    sgn = np.where(np.arange(HD) % 2 == 0, -1.0, 1.0).astype(np.float32)[:, None]
    sin_ext = np.repeat(sin, 2, axis=1).T * sgn  # sin'[d, s]
    rqq_np = np.concatenate(
        [np.tile(cos_ext, (2, 1)), np.tile(sin_ext, (2, 1))], axis=1
    ).astype(np.float16)  # [128, 4096] full scale
    idf_np = np.zeros((128, 64), dtype=np.float32)
    idf_np[64:128] = np.eye(64, dtype=np.float32)
    tri_np = np.concatenate(
        [
            (np.arange(128)[:, None] <= np.arange(128)[None, :]).astype(np.float16),
            np.eye(128, dtype=np.float16),
        ],
        axis=1,
    )  # [128, 256]: triangle | identity

    in_maps = []
    for i in range(NCORES):
        scale = 1.0 / np.sqrt(np.float32(HD))
        wl = np.concatenate(
            [
                wqkv[256 * i : 256 * i + 256] * scale,
                wqkv[D + 64 * i : D + 64 * i + 64],
                wqkv[D + KV_SIZE + 64 * i : D + KV_SIZE + 64 * i + 64],
            ],
            axis=0,
        )  # [384, D]
        wq_sb = _pack16(np.ascontiguousarray(wl.T)).astype(np.float16)
        woT = np.ascontiguousarray(wo[:, 256 * i : 256 * i + 256].T)  # [256, D]
        wo_sb = (
            np.ascontiguousarray(woT.reshape(2, 128, D).transpose(1, 0, 2))
            .reshape(128, 2 * D)
            .astype(np.float16)
        )
        in_maps.append(
            {
                "hT": hT_sb,
                "wq": wq_sb,
                "wo": wo_sb,
                "rqq": rqq_np,
                "tri": tri_np,
                "idf": idf_np,
            }
        )
    return in_maps


def run_on_hw(in_maps, trace=False, **kw):
    from concourse.bass_utils import run_bass_kernel_spmd

    nc = get_module()
    return run_bass_kernel_spmd(nc, in_maps, list(range(NCORES)), trace=trace, **kw)


def kernel(hidden_states, freqs_cis, wqkv, wo):
    in_maps = prep_inputs(hidden_states, freqs_cis, wqkv, wo)
    res = run_on_hw(in_maps)
    acc = np.zeros((S, D), dtype=np.float64)
    for r in res.results:
        acc += np.asarray(r["out"], dtype=np.float64)
    return acc.astype(np.float32).reshape(1, S, D)


# revision 60
# speedup vs baseline: 1.0621x; 1.0452x over previous
"""Tensor-parallel (over GQA head groups) multi-head attention for 8 trn2 cores.

Each core owns 4 query heads + their shared kv head (one GQA group), the
matching 384 rows of wqkv and 256 columns of wo.  Every core computes a full
[S, D] partial of the output projection; the host sums the 8 partials.

v2: fp16 data end-to-end (host converts; PSUM stays fp32) and a software-
pipelined schedule that keeps the PE busy continuously:
  - projection runs in four 512-column quarters, each as three m-serial
    passes (q01/q23/kv) over 16 resident hT chunks -> only 2 PSUM banks,
    so projection overlaps attention in PSUM;
  - attention chunk c is emitted between projection quarters c+1/c+2;
    out-projection chunks are emitted after the projection pools close;
  - scores on diagonal blocks restrict the moving operand to the live
    triangle columns (128-granular staircase);
  - the softmax denominator broadcast reuses the evacuated po psum bank
    (ones-row matmul per bank, then a lane-aligned reciprocal);
  - exp is computed as exp(s - 4) so the unnormalized f16 sums stay in
    range (softmax is shift-invariant);
  - v is kept in f32 (second half of the kv evacuation) and transposed
    f32r through the projection psum ring.
Dataflow inside one core otherwise as v1 (scores transposed [ks, qs], ones
column in V for the denominator).
"""

import sys

if "/opt/trn_rl_repo" not in sys.path:
    sys.path.insert(0, "/opt/trn_rl_repo")

import numpy as np

S = 2048
D = 2048
HD = 64
N_HEAD = 32
N_KV = 8
NCORES = 8
QH_PER_CORE = N_HEAD // NCORES  # 4
KV_SIZE = N_KV * HD  # 512

_CACHE = {}


def _build_module(debug=False):
    from contextlib import ExitStack

    import concourse.mybir as mybir
    import concourse.tile as tile
    from concourse import bacc
    from concourse.bass import ds

    FP = mybir.dt.float32
    F16 = mybir.dt.float16
    EXP = mybir.ActivationFunctionType.Exp

    nc = bacc.Bacc(
        "TRN2",
        target_bir_lowering=False,
        debug=False,
        enable_asserts=False,
        num_devices=NCORES,
    )
    # register the exp bias constant (softmax shift, see emit_expav)
    _c = nc.alloc_sbuf_tensor("const-float32-neg4", [128, 1], FP)
    nc.gpsimd.memset(_c.ap(), -4.0)
    nc.const_aps.aps[(FP, -4.0)] = _c.ap()
    nc.all_engine_barrier()

    # [p, 2048*g + s] = hidden[s, 128*g + p]
    hT = nc.dram_tensor("hT", [128, 16 * S], F16, kind="ExternalInput").ap()
    # [p, 384*g + r] = wqkv_local[r, 128*g + p]; r: 0-255 q, 256-319 k, 320-383 v
    wq = nc.dram_tensor("wq", [128, 16 * 384], F16, kind="ExternalInput").ap()
    # [p, 2048*u + e] = wo[e, 256*core + 128*u + p]
    wo = nc.dram_tensor("wo", [128, 2 * 2048], F16, kind="ExternalInput").ap()
    # rope tables, full scale (the q weights carry 1/sqrt(hd)):
    # cols 0:2048 cos, 2048:4096 sin'
    rqq = nc.dram_tensor("rqq", [128, 2 * S], F16, kind="ExternalInput").ap()
    # cols 0:128 tri[p, f] = (p <= f); cols 128:256 identity[p, f] = (p == f)
    tri = nc.dram_tensor("tri", [128, 256], F16, kind="ExternalInput").ap()
    # f32 identity in rows 64-127 (rhs of the f32r v transposes)
    idf = nc.dram_tensor("idf", [128, 64], mybir.dt.float32r, kind="ExternalInput").ap()
    out = nc.dram_tensor("out", [S, D], F16, kind="ExternalOutput").ap()
    if debug:
        dbg = {
            name: nc.dram_tensor(f"dbg_{name}", shape, F16, kind="ExternalOutput").ap()
            for name, shape in (
                ("q01s", [128, S]),
                ("q23s", [128, S]),
                ("ks", [128, S]),
                ("v65", [128, 16 * 65]),
                ("o2a", [128, S]),
                ("o2b", [128, S]),
                ("ex0", [128, 8 * 1024]),
                ("sc0", [128, 8 * 1024]),
                ("po0", [65, 1024]),
                ("poc0", [65, 1024]),
                ("rbr0", [64, 1024]),
            )
        }

    with tile.TileContext(nc) as tc, ExitStack() as ctx:
        const = ctx.enter_context(tc.tile_pool(name="const", bufs=1))
        wqt = [
            const.tile([128, 4 * 384], F16, tag=f"wq{k}", name=f"wq_sb{k}")
            for k in range(4)
        ]

        def wq_ap(g, m):
            return wqt[g // 4][:, ds(384 * (g % 4) + 128 * m, 128)]

        F32R = mybir.dt.float32r
        wo_sb = const.tile([128, 4096], F16, tag="wo")
        rqq_sb = const.tile([128, 4096], F16, tag="rqq")
        tri_sb = const.tile([128, 256], F16, tag="tri")
        idf_sb = const.tile([128, 64], FP, tag="idf")
        ident64 = idf_sb[64:128, :].bitcast(F32R)

        qsw = ctx.enter_context(tc.tile_pool(name="qsw", bufs=1))
        q01s = qsw.tile([128, S], F16, tag="q01s")
        q23s = qsw.tile([128, S], F16, tag="q23s")
        ks = qsw.tile([128, S], F16, tag="ks")
        v_sb = qsw.tile([128, 16 * 65], F16, tag="v")
        o2a = qsw.tile([128, S], F16, tag="o2a")
        o2b = qsw.tile([128, S], F16, tag="o2b")
        v65 = v_sb.rearrange("p (j c) -> p j c", c=65)

        # attention pools (live through the whole kernel)
        expool = ctx.enter_context(tc.tile_pool(name="ex", bufs=4))
        rspool = ctx.enter_context(tc.tile_pool(name="rs", bufs=2))
        rcpool = ctx.enter_context(tc.tile_pool(name="rc", bufs=2))
        rbpool = ctx.enter_context(tc.tile_pool(name="rb", bufs=2))
        nmpool = ctx.enter_context(tc.tile_pool(name="nm", bufs=2))
        # attention psum pools are created after quarter 0 releases its
        # 3-bank ring (right side of the arena; release is LIFO per side)
        phaseA = ExitStack()
        psS = psO = None

        # projection-phase pools (close before the out-projection opens)
        phaseP = ExitStack()
        hpool = phaseP.enter_context(tc.tile_pool(name="hp", bufs=24))
        qraw = phaseP.enter_context(tc.tile_pool(name="qraw", bufs=1))
        q01 = qraw.tile([128, S], F16, tag="q01")
        q23 = qraw.tile([128, S], F16, tag="q23")
        kv = qraw.tile([128, S], F16, tag="kv")  # rows 0:64 = k (rope input)
        vraw = qraw.tile([128, S], FP, tag="vraw")  # rows 64:128 = v, f32
        scpool = phaseP.enter_context(tc.tile_pool(name="sc", bufs=4))
        phase0 = ExitStack()
        psA0 = phase0.enter_context(tc.tile_pool(name="psA0", bufs=1, space="PSUM"))
        psA = None

        SWAP_MASK = [i ^ 1 for i in range(32)]

        def rope_quarter(dst, raw, p, costab, sintab, q, nm):
            # dst = raw * cos + pairswap(raw) * sin' on [0:p, 512q:512q+512]
            cs = ds(512 * q, 512)
            sw = scpool.tile([128, 512], F16, tag="sc", name=f"sw_{nm}{q}")
            nc.vector.stream_shuffle(sw[0:p, :], raw[0:p, cs], SWAP_MASK)
            t0 = scpool.tile([128, 512], F16, tag="sc", name=f"t0_{nm}{q}")
            nc.vector.tensor_mul(t0[0:p, :], raw[0:p, cs], costab[0:p, cs])
            nc.vector.tensor_mul(sw[0:p, :], sw[0:p, :], sintab[0:p, cs])
            nc.vector.tensor_add(dst[0:p, cs], t0[0:p, :], sw[0:p, :])

        # global DMA plan: one ordered stream of large transfers.  hT comes as
        # 2048-column quads ([128, 4, 512] strided, >=1KB contiguous runs) so
        # each DMA is bus-bound, not HWDGE-dispatch-bound (625ns fixed cost).
        hT_r = hT.rearrange("p (g s) -> p g s", s=2048)
        rq_r = rqq.rearrange("p (h s) -> p h s", s=2048)
        hq_tiles = {}

        def dma_hq(q, t, half=None):
            # quad t of quarter q: g-chunks 4t..4t+3, columns 512q..512q+512
            if half is None:
                gs, n = 4 * t, 4
            else:
                gs, n = 4 * t + 2 * half, 2
            tile = hq_tiles.get((q, t))
            if tile is None:
                tile = hpool.tile([128, 4, 512], F16, tag="hc", name=f"hq_{q}_{t}")
                hq_tiles[(q, t)] = tile
            nc.sync.dma_start(
                tile[:, gs - 4 * t : gs - 4 * t + n, :],
                hT_r[:, gs : gs + n, ds(512 * q, 512)],
            )

        def hc_ap(q, g):
            return hq_tiles[(q, g // 4)][:, g % 4, :]

        rqsb_r = rqq_sb.rearrange("p (h s) -> p h s", s=2048)

        def dma_rq(q):
            nc.sync.dma_start(
                rqsb_r[:, :, ds(512 * q, 512)], rq_r[:, :, ds(512 * q, 512)]
            )

        dma_plan = [
            ("wq0a", lambda: nc.sync.dma_start(wqt[0][:, 0:384], wq[:, 0:384])),
            ("hq00a", lambda: dma_hq(0, 0, 0)),
            ("wq0b", lambda: nc.sync.dma_start(wqt[0][:, 384:1536], wq[:, 384:1536])),
            ("hq00b", lambda: dma_hq(0, 0, 1)),
            ("wq1", lambda: nc.sync.dma_start(wqt[1][:], wq[:, ds(1536, 1536)])),
            ("hq01", lambda: dma_hq(0, 1)),
            ("wq2", lambda: nc.sync.dma_start(wqt[2][:], wq[:, ds(3072, 1536)])),
            ("hq02", lambda: dma_hq(0, 2)),
            ("wq3", lambda: nc.sync.dma_start(wqt[3][:], wq[:, ds(4608, 1536)])),
            ("hq03", lambda: dma_hq(0, 3)),
            ("rq0", lambda: dma_rq(0)),
            ("tri", lambda: (nc.sync.dma_start(tri_sb[:], tri),
                             nc.sync.dma_start(idf_sb[:].bitcast(F32R), idf))),
            ("hq10", lambda: dma_hq(1, 0)),
            ("hq11", lambda: dma_hq(1, 1)),
            ("hq12", lambda: dma_hq(1, 2)),
            ("rq1", lambda: dma_rq(1)),
            ("hq13", lambda: dma_hq(1, 3)),
            ("hq20", lambda: dma_hq(2, 0)),
            ("wo0", lambda: nc.sync.dma_start(wo_sb[:, 0:2048], wo[:, 0:2048])),
            ("hq21", lambda: dma_hq(2, 1)),
            ("hq22", lambda: dma_hq(2, 2)),
            ("wo1", lambda: nc.sync.dma_start(wo_sb[:, 2048:4096], wo[:, 2048:4096])),
            ("rq2", lambda: dma_rq(2)),
            ("hq23", lambda: dma_hq(2, 3)),
            ("hq30", lambda: dma_hq(3, 0)),
            ("hq31", lambda: dma_hq(3, 1)),
            ("hq32", lambda: dma_hq(3, 2)),
            ("rq3", lambda: dma_rq(3)),
            ("hq33", lambda: dma_hq(3, 3)),
        ]
        plan_pos = {key: idx for idx, (key, _) in enumerate(dma_plan)}
        feed_state = {"next": 0}

        def feed_until(key):
            stop = plan_pos[key] + 1
            while feed_state["next"] < stop:
                dma_plan[feed_state["next"]][1]()
                feed_state["next"] += 1

        def emit_vtrans(q, vtp):
            # v transposes for quarter q: f32r through the projection psum
            # ring slot, then one f16 evacuation into v65
            vt16 = None
            vt = vtp.tile([128, 512], FP, tag=vtp_tag(vtp), name=f"vt_{q}")
            for jj in range(4):
                j = 4 * q + jj
                nc.tensor.transpose(
                    vt[:, ds(64 * jj, 64)].bitcast(F32R),
                    vraw[64:128, ds(128 * j, 128)].bitcast(F32R),
                    ident64,
                )
            nc.vector.tensor_copy(
                v65[:, 4 * q : 4 * q + 4, 0:64],
                vt[:, 0:256].rearrange("p (jj c) -> p jj c", c=64),
            )
            nc.vector.tensor_copy(
                v65[:, 4 * q : 4 * q + 4, 64:65],
                tri_sb[:, 127:128][:, None, :].to_broadcast([128, 4, 1]),
            )

        def vtp_tag(pool):
            return "p0" if pool is psA0 else "pj"

        def gen_proj_quarter0():
            # quarter 0 runs all three m-passes interleaved at the pace the
            # DMA stream can sustain (the front is inherently supply-bound)
            pts = [
                psA0.tile([128, 512], FP, tag=f"p{m}", name=f"pj0_{m}")
                for m in range(3)
            ]
            feed_until("hq00b")
            for g in range(12):
                feed_until(dma_plan[min(g + 4, 13)][0])
                for m in range(3):
                    nc.tensor.matmul(
                        pts[m][:], wq_ap(g, m), hc_ap(0, g),
                        start=(g == 0), stop=False,
                    )
                yield 639
            # finish the three passes m-serially, kv first: rope-k (which
            # gates the first attention scores) starts ~3us earlier
            cos_t, sin_t = rqq_sb[:, 0:2048], rqq_sb[:, 2048:4096]
            for m, dst, dsts, p, nm in (
                (2, kv, ks, 64, "k"),
                (0, q01, q01s, 128, "q01"),
                (1, q23, q23s, 128, "q23"),
            ):
                feed_until("hq11")
                for g in range(12, 16):
                    nc.tensor.matmul(
                        pts[m][:], wq_ap(g, m), hc_ap(0, g),
                        start=False, stop=(g == 15),
                    )
                if m == 2:
                    nc.scalar.copy(kv[0:64, 0:512], pts[m][0:64, :])
                    nc.scalar.copy(vraw[64:128, 0:512].bitcast(F32R), pts[m][64:128, :])
                else:
                    nc.scalar.copy(dst[:, 0:512], pts[m][:])
                rope_quarter(dsts, dst, p, cos_t, sin_t, 0, nm)
                if m == 2:
                    nc.sync.dma_start(ks[64:128, 0:512], ks[0:64, 0:512])
                yield 852
            emit_vtrans(0, psA0)
            yield 80
            feed_until("hq11")
            yield 120

        def gen_proj_quarter(q):
            feed_until(f"hq{q}1")
            pts = {}
            for m in (0, 1):
                pts[m] = psA.tile([128, 512], FP, tag="pj", name=f"pj_{q}_{m}")
            for g in range(16):
                if g % 4 == 2 and g < 12:
                    feed_until(f"hq{q}{g // 4 + 2}" if g // 4 + 2 <= 3 else f"hq{q}3")
                for m in (0, 1):
                    nc.tensor.matmul(
                        pts[m][:], wq_ap(g, m), hc_ap(q, g),
                        start=(g == 0), stop=(g == 15),
                    )
                yield 426
            cs = ds(512 * q, 512)
            nc.scalar.copy(q01[:, cs], pts[0][:])
            rope_quarter(q01s, q01, 128, rqq_sb[:, 0:2048], rqq_sb[:, 2048:4096], q, "q01")
            yield 300
            nc.scalar.copy(q23[:, cs], pts[1][:])
            rope_quarter(q23s, q23, 128, rqq_sb[:, 0:2048], rqq_sb[:, 2048:4096], q, "q23")
            yield 300
            pt2 = psA.tile([128, 512], FP, tag="pj", name=f"pj_{q}_2")
            for g in range(16):
                nc.tensor.matmul(
                    pt2[:], wq_ap(g, 2), hc_ap(q, g),
                    start=(g == 0), stop=(g == 15),
                )
                if g % 4 == 3:
                    # deep-prefetch the next quarter while the psum ring is
                    # the only DMA consumer
                    if q < 3:
                        feed_until(f"hq{q + 1}{min(g // 4, 3)}")
                    yield 852
            nc.scalar.copy(kv[0:64, cs], pt2[0:64, :])
            nc.scalar.copy(vraw[64:128, cs].bitcast(F32R), pt2[64:128, :])
            rope_quarter(ks, kv, 64, rqq_sb[:, 0:2048], rqq_sb[:, 2048:4096], q, "k")
            yield 300
            # duplicate rotated k at partitions 64-127 (odd heads' score
            # matmuls read lhsT/rhs both at base 64)
            nc.sync.dma_start(ks[64:128, cs], ks[0:64, cs])
            emit_vtrans(q, psA)
            if q < 3:
                feed_until(f"hq{q + 1}1")
            yield 120

        def gen_attention_chunk(c):
            nj = 4 * c + 4
            for hp in range(2):
                po = psO.tile([65, 1024], FP, tag="po", name=f"po_{c}_{hp}")

                def emit_scores(j):
                    r = j - 4 * c  # >= 0 on diagonal blocks
                    off = 128 * r if r >= 0 else 0
                    ps = psS.tile([128, 1024], FP, tag="ps", name=f"ps_{c}_{hp}_{j}")
                    for hh in range(2):
                        h = 2 * hp + hh
                        qt = q01s if h < 2 else q23s
                        base = 64 * (h % 2)
                        nc.tensor.matmul(
                            ps[:, ds(512 * hh + off, 512 - off)],
                            ks[base : base + 64, ds(128 * j, 128)],
                            qt[base : base + 64, ds(512 * c + off, 512 - off)],
                        )
                    return ps, off, r >= 0

                def emit_expav(j, ps, off, diag):
                    # exp(s - 4): softmax is shift-invariant and the bias
                    # keeps the unnormalized f16 sums (up to ~exp(9) * |v|)
                    # well inside f16 range
                    ex = expool.tile([128, 1024], F16, tag="ex", name=f"ex_{c}_{hp}_{j}")
                    if not diag:
                        nc.scalar.activation(ex[:], ps[:], EXP, bias=-4.0)
                    else:
                        w = 512 - off
                        psv = ps.rearrange("p (h w) -> p h w", w=512)[:, :, ds(off, w)]
                        exv = ex.rearrange("p (h w) -> p h w", w=512)[:, :, ds(off, w)]
                        nc.scalar.activation(exv, psv, EXP, bias=-4.0)
                        exd = ex.rearrange("p (h w) -> p h w", w=512)[:, :, ds(off, 128)]
                        nc.vector.tensor_mul(
                            exd,
                            exd,
                            tri_sb[:, 0:128][:, None, :].to_broadcast([128, 2, 128]),
                        )
                    if debug and c == 0:
                        sl = ds(1024 * (4 * hp + j), 1024)
                        nc.sync.dma_start(dbg["ex0"][:, sl], ex[:])
                        sc16 = expool.tile(
                            [128, 1024], F16, tag="ex", name=f"scd_{hp}_{j}"
                        )
                        nc.vector.tensor_copy(sc16[:], ps[:])
                        nc.sync.dma_start(dbg["sc0"][:, sl], sc16[:])
                    for hh in range(2):
                        nc.tensor.matmul(
                            po[0:65, ds(512 * hh + off, 512 - off)],
                            v_sb[:, ds(65 * j, 65)],
                            ex[:, ds(512 * hh + off, 512 - off)],
                            start=(j == 0),
                            stop=(j == nj - 1),
                            skip_group_check=True,
                        )

                # one-j lookahead: scores(j+1) land on the PE between
                # scores(j) and av(j) so the exp never stalls the PE
                def jcost(j):
                    # PE ns of one scores OR av pair at this block's trim
                    r = j - 4 * c
                    off = 128 * r if r >= 0 else 0
                    return int((512 - off) * 0.833)

                prev = emit_scores(0)
                for j in range(1, nj):
                    cur = emit_scores(j)
                    emit_expav(j - 1, *prev)
                    prev = cur
                    yield jcost(j) + jcost(j - 1)
                emit_expav(nj - 1, *prev)
                yield jcost(nj - 1)
                # evacuate the accumulator so the bank frees for the next
                # head pair, then normalize: reciprocal of the sums row,
                # partition_broadcast on the (idle) Pool engine, two muls
                poc = rspool.tile([65, 1024], F16, tag="rs", name=f"poc_{c}_{hp}")
                if debug and c == 0 and hp == 0:
                    pod = rspool.tile([65, 1024], F16, tag="pod", name="pod")
                    nc.vector.tensor_copy(pod[:], po[:])
                    nc.sync.dma_start(dbg["po0"], pod[:])
                nc.scalar.copy(poc[:, 512:1024], po[:, 512:1024])
                nc.vector.tensor_copy(poc[:, 0:512], po[:, 0:512])
                rbr = rbpool.tile([64, 1024], F16, tag="rbr", name=f"rbr_{c}_{hp}")
                dsttile = o2a if hp == 0 else o2b
                nm = nmpool.tile([64, 512], F16, tag="nm", name=f"nm_{c}_{hp}")
                # broadcast the sums row back into the (already-evacuated)
                # po bank with a ones-row matmul -- ones at base 64 to match
                # poc's denominator row -- then a lane-aligned reciprocal
                for half in (1, 0):
                    hs = ds(512 * half, 512)
                    nc.tensor.matmul(
                        po[0:64, hs], tri_sb[64:65, 64:128], poc[64:65, hs],
                        start=True, stop=True,
                    )
                    with nc.allow_low_precision(reason="softmax denom recip f16"):
                        nc.vector.reciprocal(rbr[0:64, hs], po[0:64, hs])
                    if half == 0:
                        nc.vector.tensor_mul(
                            dsttile[0:64, ds(512 * c, 512)],
                            poc[0:64, hs],
                            rbr[0:64, hs],
                        )
                    else:
                        nc.vector.tensor_mul(nm[0:64, :], poc[0:64, hs], rbr[0:64, hs])
                nc.sync.dma_start(dsttile[64:128, ds(512 * c, 512)], nm[0:64, :])
                if debug and c == 0 and hp == 0:
                    nc.sync.dma_start(dbg["poc0"], poc[:])
                    nc.sync.dma_start(dbg["rbr0"], rbr[0:64, :])
                yield 60

        post = {}

        def open_post_pools():
            post["ost"] = ctx.enter_context(tc.tile_pool(name="ost", bufs=6))
            post["psP"] = ctx.enter_context(tc.tile_pool(name="psP", bufs=2, space="PSUM"))

        def gen_outproj_chunk(c, tail=False, pskey="psP", bs=range(4)):
            for b in bs:
                for n2 in range(2):  # pairs of 512-wide e-slices -> one DMA
                    st = post["ost"].tile(
                        [128, 1024], F16, tag="st", name=f"st_{c}_{b}_{n2}"
                    )
                    for nn in range(2):
                        n = 2 * n2 + nn
                        pp = post[pskey].tile(
                            [128, 512], FP, tag="pp", name=f"pp_{c}_{b}_{n}"
                        )
                        nc.tensor.matmul(
                            pp[:],
                            o2a[:, ds(512 * c + 128 * b, 128)],
                            wo_sb[:, ds(512 * n, 512)],
                            start=True,
                            stop=False,
                        )
                        nc.tensor.matmul(
                            pp[:],
                            o2b[:, ds(512 * c + 128 * b, 128)],
                            wo_sb[:, ds(2048 + 512 * n, 512)],
                            start=False,
                            stop=True,
                        )
                        # in the pure-PE tail alternate evacuation engines so
                        # the psum ring keeps pace with the matmuls
                        if tail and nn == 1:
                            nc.scalar.copy(st[:, ds(512, 512)], pp[:])
                        else:
                            nc.vector.tensor_copy(st[:, ds(512 * nn, 512)], pp[:])
                        yield 426
                    nc.sync.dma_start(
                        out[ds(128 * (4 * c + b), 128), ds(1024 * n2, 1024)], st[:]
                    )

        def chain(*gens):
            for g in gens:
                yield from g

        def closer():
            phaseP.close()
            open_post_pools()
            return
            yield  # pragma: no cover

        def weave(ga, gb, wa=1.0, wb=1.0):
            # proportional-progress interleave of two emission streams:
            # step the stream with the smaller fraction-complete so a short
            # filler spreads across the whole window instead of front-loading
            ta = tb = 0.0
            da = db = False
            while not (da and db):
                if db or (not da and ta / wa <= tb / wb):
                    try:
                        ta += next(ga)
                    except StopIteration:
                        da = True
                else:
                    try:
                        tb += next(gb)
                    except StopIteration:
                        db = True

        def run(g):
            for _ in g:
                pass

        # ---- pipeline: P0 [P1|A0] [P2|A1] [P3,close,O0|A2] [O1,O2|A3] O3 --
        run(gen_proj_quarter0())
        phase0.close()
        psA = phaseP.enter_context(tc.tile_pool(name="psA", bufs=2, space="PSUM"))
        psS = phaseA.enter_context(
            tc.tile_pool(name="psS", bufs=2, space="PSUM", side="right")
        )
        psO = phaseA.enter_context(
            tc.tile_pool(name="psO", bufs=1, space="PSUM", side="right")
        )
        weave(gen_proj_quarter(1), gen_attention_chunk(0))
        weave(gen_proj_quarter(2), gen_attention_chunk(1))
        weave(
            chain(gen_proj_quarter(3), closer(), gen_outproj_chunk(0)),
            gen_attention_chunk(2),
            wa=17.5,
            wb=16.6,
        )
        weave(
            chain(gen_outproj_chunk(1), gen_outproj_chunk(2, bs=range(3))),
            gen_attention_chunk(3),
            wa=11.9,
            wb=22.1,
        )
        # attention psum freed -> deep out-proj ring; the O2 remainder hides
        # the last normalize chain before O3 starts
        phaseA.close()
        post["psP2"] = ctx.enter_context(
            tc.tile_pool(name="psP2", bufs=4, space="PSUM", side="right")
        )
        run(gen_outproj_chunk(2, tail=True, pskey="psP2", bs=range(3, 4)))
        run(gen_outproj_chunk(3, tail=True, pskey="psP2"))
        if debug:
            for name, tile in (
                ("q01s", q01s), ("q23s", q23s), ("ks", ks),
                ("v65", v_sb), ("o2a", o2a), ("o2b", o2b),
            ):
                nc.sync.dma_start(dbg[name], tile[:])

    nc.compile()
    return nc


def get_module(debug=False):
    key = ("nc", debug)
    if key not in _CACHE:
        _CACHE[key] = _build_module(debug=debug)
    return _CACHE[key]


def _pack16(x):
    # [16*128, N] -> [128, 16*N] with [p, N*g + n] = x[128*g + p, n]
    n = x.shape[1]
    return (
        np.ascontiguousarray(x.reshape(16, 128, n).transpose(1, 0, 2)).reshape(128, 16 * n)
    )


def prep_inputs(hidden_states, freqs_cis, wqkv, wo):
    h = np.asarray(hidden_states, dtype=np.float32)[0]  # [S, D]
    fc = np.asarray(freqs_cis, dtype=np.float32)  # [S, 32, 2]
    wqkv = np.asarray(wqkv, dtype=np.float32)  # [3072, D]
    wo = np.asarray(wo, dtype=np.float32)  # [D, D]

    hT_sb = _pack16(np.ascontiguousarray(h.T)).astype(np.float16)

    cos = fc[:, :, 0]  # [S, 32]
    sin = fc[:, :, 1]
    cos_ext = np.repeat(cos, 2, axis=1).T  # [64, S]
    sgn = np.where(np.arange(HD) % 2 == 0, -1.0, 1.0).astype(np.float32)[:, None]
    sin_ext = np.repeat(sin, 2, axis=1).T * sgn  # sin'[d, s]
    rqq_np = np.concatenate(
        [np.tile(cos_ext, (2, 1)), np.tile(sin_ext, (2, 1))], axis=1
    ).astype(np.float16)  # [128, 4096] full scale
    idf_np = np.zeros((128, 64), dtype=np.float32)
    idf_np[64:128] = np.eye(64, dtype=np.float32)
    tri_np = np.concatenate(
        [
            (np.arange(128)[:, None] <= np.arange(128)[None, :]).astype(np.float16),
            np.eye(128, dtype=np.float16),
        ],
        axis=1,
    )  # [128, 256]: triangle | identity

    in_maps = []
    for i in range(NCORES):
        scale = 1.0 / np.sqrt(np.float32(HD))
        wl = np.concatenate(
            [
                wqkv[256 * i : 256 * i + 256] * scale,
                wqkv[D + 64 * i : D + 64 * i + 64],
                wqkv[D + KV_SIZE + 64 * i : D + KV_SIZE + 64 * i + 64],
            ],
            axis=0,
        )  # [384, D]
        wq_sb = _pack16(np.ascontiguousarray(wl.T)).astype(np.float16)
        woT = np.ascontiguousarray(wo[:, 256 * i : 256 * i + 256].T)  # [256, D]
        wo_sb = (
            np.ascontiguousarray(woT.reshape(2, 128, D).transpose(1, 0, 2))
            .reshape(128, 2 * D)
            .astype(np.float16)
        )
        in_maps.append(
            {
                "hT": hT_sb,
                "wq": wq_sb,
                "wo": wo_sb,
                "rqq": rqq_np,
                "tri": tri_np,
                "idf": idf_np,
            }
        )
    return in_maps


def run_on_hw(in_maps, trace=False, **kw):
    from concourse.bass_utils import run_bass_kernel_spmd

    nc = get_module()
    return run_bass_kernel_spmd(nc, in_maps, list(range(NCORES)), trace=trace, **kw)


def kernel(hidden_states, freqs_cis, wqkv, wo):
    in_maps = prep_inputs(hidden_states, freqs_cis, wqkv, wo)
    res = run_on_hw(in_maps)
    acc = np.zeros((S, D), dtype=np.float64)
    for r in res.results:
        acc += np.asarray(r["out"], dtype=np.float64)
    return acc.astype(np.float32).reshape(1, S, D)
